# revision 18
# baseline (speedup 1.0000x reference)
"""Trainium2 Bass kernel for nn_BlocksCore (RIMs-style BlocksCore forward).

Sharding: data-parallel over batch B=2048 across 8 NeuronCores (256 rows each,
zero cross-core communication; all model ops are batch-independent).

Key optimizations over the bf16 baseline:
  - W2 = Wv1[1] @ Wih^T is folded on the host, so the GRU input projection
    gi = s * (inp @ W2) consumes inp directly: the whole v1 matmul stage
    disappears and one fp8 quantization step is avoided.
  - gi and gh run as fp8e4 DoubleRow matmuls (2 k-subtiles per pass). Both
    arrive in PSUM scaled by 2^14 (inp*16 x W2*4096 / 16, hx*16 x Whh*1024),
    so s*gi+gh combines with one STT and the 2^-14 descale rides the
    sigmoid/tanh activation scale for free.
  - Routing stays fp32 (top-k margins are ~1e-6) but restructured: k1 and
    q = hx@Wq1 are computed row-major on the PE (half the fp32 columns of
    the old k1T/wp form) and the 64-dim dot runs on DVE.
  - bt-major pipeline: all 8 blocks of batch-half 0 run first, then its
    comm-attention chain (Vector-heavy) is emitted so it overlaps with
    batch-half 1's GRU matmuls on the PE.
  - fc and gate projections share one block-diagonal stationary matrix
    (fgw2), so each q-pair needs a single K=128 matmul instead of two
    half-wasted K=64 matmuls.
  - Elementwise work is spread V/S/G: STT gate-combines and small TTs on
    Vector, sigmoid/tanh/copies on Scalar, h' blend products on GpSimd.
"""
import sys

sys.path.insert(0, "/opt/trn_rl_repo")

import numpy as np
import ml_dtypes

import concourse.bass as bass
import concourse.tile as tile
from concourse.masks import make_identity
from concourse import bacc, mybir

f32 = mybir.dt.float32
bf16 = mybir.dt.bfloat16
fp8 = mybir.dt.float8e4
AF = mybir.ActivationFunctionType
ALU = mybir.AluOpType
AX = mybir.AxisListType
DR = mybir.MatmulPerfMode.DoubleRow

B, NINP, NHID = 2048, 1024, 2048
NB, BS, G3 = 8, 256, 768          # blocks, block_size_out, 3*BS
NH2, DK2, DV2, HD = 4, 16, 16, 64  # comm attn heads, dims, NH2*DV2
NCORES = 8
BL = B // NCORES                   # 256 rows per core
KI_IN = NINP // 128                # 8
KI_HID = NHID // 128               # 16

BF = ml_dtypes.bfloat16
F8 = ml_dtypes.float8_e4m3

# power-of-two quantization scales (values clipped to +-240 on host)
S_INP = 16.0      # inp, hx fp8 copies
S_W2 = 1024.0     # W2 = Wv1 @ Wih^T
S_WHH = 1024.0
# pu = (inp*16) @ (W2*1024) = gi * 2^14 ; pvh = (hx*16) @ (Whh*1024) = gh * 2^14
PSC = 2.0 ** -14  # descale applied inside sigmoid/tanh activation scale


def _vap(sl, dims):
    """Custom free-dim view: keep partition dim of slice `sl`, replace free dims."""
    return bass.AP(sl.tensor, sl.offset, [sl.ap[0]] + [list(d) for d in dims])


def _build(has_gru_bias: bool):
    nc = bacc.Bacc("TRN2", target_bir_lowering=False, debug=False,
                   num_devices=NCORES)

    def din(name, shape, dt):
        return nc.dram_tensor(name, list(shape), dt, kind="ExternalInput").ap()

    inpT_d = din("inpT", (128, KI_IN * BL), f32)          # [p, ki*256+c]
    inpT8_d = din("inpT8", (128, 2 * KI_IN * 128), fp8)   # [p, bt*1024+ki*128+c]
    hxP_d = din("hxP", (128, 2 * NHID), f32)              # [p, bt*2048+f]
    hxT8_d = din("hxT8", (128, 2 * KI_HID * 128), fp8)    # [p, bt*2048+t*128+c]
    hxTf_d = din("hxTf", (128, 2 * KI_HID * 128), f32)    # [p, bt*2048+t*128+c]
    wk1_d = din("wk1", (128, KI_IN * 64), f32)            # [p, ki*64+d]
    wq1_d = din("wq1", (128, KI_HID * 64), f32)           # [p, t*64+d]
    w28_d = din("w28", (128, NB * KI_IN * G3), fp8)       # [p, blk*6144+ki*768+g]
    whh_d = din("whh", (128, NB * 2 * G3), fp8)           # [p, blk*1536+ki*768+g]
    wqkv_d = din("wqkv", (128, NB * 2 * 192), bf16)       # [p, t*192 + {q2|k2/4|v2}]
    fgw2_d = din("fgw2", (128, 1024), bf16)               # block-diag [fc|gate; fc|gate]
    if has_gru_bias:
        bbB_d = din("bbB", (128, NB * G3), f32)           # (bih+bhh)*2^14 bcast
        ones_d = din("onesrow", (1, 128), bf16)
        fgb2_d = din("fgb2", (1, 1024), bf16)             # fc_b|gate_b twice
    out_d = nc.dram_tensor("out", [2, 128, NHID], f32, kind="ExternalOutput").ap()
    mask_d = nc.dram_tensor("maskout", [128, 16], f32, kind="ExternalOutput").ap()

    from contextlib import ExitStack
    with tile.TileContext(nc) as tc, ExitStack() as ctx:
        P = ctx.enter_context(tc.tile_pool(name="persist", bufs=1))
        p32 = ctx.enter_context(tc.tile_pool(name="p32", bufs=2))
        hxtp = ctx.enter_context(tc.tile_pool(name="hxtp", bufs=1))
        gwork = ctx.enter_context(tc.tile_pool(name="gwork", bufs=2))
        prodp = ctx.enter_context(tc.tile_pool(name="prodp", bufs=1))
        tailp = ctx.enter_context(tc.tile_pool(name="tailp", bufs=2))
        # PSUM budget, bank-granular (8x2KB): pu 2x2 + pvh 1x2 + sm 1x1 + fg 1x1
        ps_u = ctx.enter_context(tc.tile_pool(name="ps_u", bufs=2, space="PSUM"))
        ps_v = ctx.enter_context(tc.tile_pool(name="ps_v", bufs=1, space="PSUM"))
        ps_sm = ctx.enter_context(tc.tile_pool(name="ps_sm", bufs=1, space="PSUM"))
        ps_fg = ctx.enter_context(tc.tile_pool(name="ps_fg", bufs=1, space="PSUM"))

        # ---- persistent sbuf tensors
        inpT8 = P.tile([128, 2 * KI_IN * 128], fp8, tag="inpT8")
        hxP_f = P.tile([128, 2 * NHID], f32, tag="hxP_f")
        hxT8 = P.tile([128, 2 * KI_HID * 128], fp8, tag="hxT8")
        wk1 = P.tile([128, KI_IN * 64], f32, tag="wk1")
        wq1 = P.tile([128, KI_HID * 64], f32, tag="wq1")
        w28 = P.tile([128, NB * KI_IN * G3], fp8, tag="w28")
        whh = P.tile([128, NB * 2 * G3], fp8, tag="whh")
        wqkv = P.tile([128, NB * 2 * 192], bf16, tag="wqkv")
        fgw2 = P.tile([128, 1024], bf16, tag="fgw2")
        k1s = P.tile([128, 2 * 64], f32, tag="k1s")
        aP = P.tile([128, 16], f32, tag="aP")
        sS = P.tile([128, 16], f32, tag="sS")
        mS = P.tile([128, 16], f32, tag="mS")
        cnt = P.tile([128, 16], f32, tag="cnt")
        cmp_t = P.tile([128, 128], f32, tag="cmp")
        hP = P.tile([128, 2 * NHID], bf16, tag="hP")
        hd = P.tile([128, 2 * NHID], bf16, tag="hd")
        hT = P.tile([128, KI_HID * BL], bf16, tag="hT")
        qk2P = P.tile([128, 2 * 1024], bf16, tag="qk2P")   # [bt*1024 + {0:q2,512:k2} + blk*64]
        v2P = P.tile([128, 2 * NB * 64], bf16, tag="v2P")  # [bt*512 + h*128 + d*8 + k]
        Lp = P.tile([128, 2 * 256], bf16, tag="Lp")
        attE = P.tile([128, 2 * 256], bf16, tag="attE")
        attS = P.tile([128, 2 * 32], bf16, tag="attS")
        attR = P.tile([128, 2 * 32], bf16, tag="attR")
        attW = P.tile([128, 2 * 256], bf16, tag="attW")
        out2P = P.tile([128, 2 * NB * 64], bf16, tag="out2P")
        out2T = P.tile([128, 4 * BL], bf16, tag="out2T")  # [(q%2)*64+hd, (q//2)*256+bt*128+b]
        identB = P.tile([128, 128], bf16, tag="identB")
        if has_gru_bias:
            bbB = P.tile([128, NB * G3], f32, tag="bbB")
            onesR = P.tile([1, 128], bf16, tag="onesR")
            fgb2 = P.tile([1, 1024], bf16, tag="fgb2")

        inpT_f = p32.tile([128, KI_IN * BL], f32, tag="big32")
        hxT_f = hxtp.tile([128, 2 * KI_HID * 128], f32, tag="hxTf")

        # ---- DMA emission. sync queue: routing-critical inp stream + fp8 GRU
        # weights; scalar queue: hx stream + attn weights.
        nc.sync.dma_start(inpT_f[:], inpT_d[:])
        nc.sync.dma_start(wk1[:], wk1_d[:])
        nc.sync.dma_start(inpT8[:], inpT8_d[:])
        nc.sync.dma_start(whh[:], whh_d[:])
        for blk in range(NB):
            nc.sync.dma_start(w28[:, blk * 6144:(blk + 1) * 6144],
                              w28_d[:, blk * 6144:(blk + 1) * 6144])
        nc.scalar.dma_start(hxT_f[:, 0:2048], hxTf_d[:, 0:2048])
        nc.scalar.dma_start(wq1[:], wq1_d[:])
        nc.scalar.dma_start(hxT8[:], hxT8_d[:])
        nc.scalar.dma_start(hxT_f[:, 2048:4096], hxTf_d[:, 2048:4096])
        nc.scalar.dma_start(hxP_f[:, 0:NHID], hxP_d[:, 0:NHID])
        nc.scalar.dma_start(hxP_f[:, NHID:], hxP_d[:, NHID:])
        nc.scalar.dma_start(wqkv[:], wqkv_d[:])
        nc.scalar.dma_start(fgw2[:], fgw2_d[:])
        if has_gru_bias:
            nc.scalar.dma_start(bbB[:], bbB_d[:])
            nc.scalar.dma_start(onesR[:], ones_d[:])
            nc.scalar.dma_start(fgb2[:], fgb2_d[:])
        make_identity(nc, identB[:])

        # ---- routing (all fp32): k1 = inp@Wk1 row-major, q = hx@Wq1 row-major,
        # a = sum_d k1*q via DVE, then sigmoid + top-k mask.
        for bt in range(2):
            kp = ps_fg.tile([128, 64], f32, tag="fg")
            for ki in range(KI_IN):
                nc.tensor.matmul(kp[:], inpT_f[:, ki * BL + bt * 128: ki * BL + (bt + 1) * 128],
                                 wk1[:, ki * 64:(ki + 1) * 64],
                                 start=(ki == 0), stop=(ki == KI_IN - 1))
            nc.scalar.activation(k1s[:, bt * 64:(bt + 1) * 64], kp[:], AF.Copy)
        for bt in range(2):
            qp = ps_fg.tile([128, 512], f32, tag="fg")
            for blk in range(NB):
                for ki in range(2):
                    t = blk * 2 + ki
                    nc.tensor.matmul(qp[:, blk * 64:(blk + 1) * 64],
                                     hxT_f[:, bt * 2048 + t * 128: bt * 2048 + (t + 1) * 128],
                                     wq1[:, t * 64:(t + 1) * 64],
                                     start=(ki == 0), stop=(ki == 1))
            prod = gwork.tile([128, 512], f32, tag="prod")
            k1bc = _vap(k1s[:, bt * 64: bt * 64 + 1], [[0, 8], [1, 64]])
            nc.vector.tensor_tensor(prod[:], qp[:], k1bc, ALU.mult)
            nc.vector.reduce_sum(aP[:, bt * 8:(bt + 1) * 8],
                                 _vap(prod[:, 0:1], [[64, 8], [1, 64]]), axis=AX.X)
        nc.scalar.activation(sS[:], aP[:], AF.Sigmoid, scale=0.125)
        # mask: cnt[bt,k] = #{j : a[bt,j] > a[bt,k]};  keep iff cnt < 4
        i0 = _vap(aP[:], [[8, 2], [1, 8], [0, 8]])
        i1 = _vap(aP[:], [[8, 2], [0, 8], [1, 8]])
        ov = _vap(cmp_t[:], [[64, 2], [1, 8], [8, 8]])
        nc.vector.tensor_tensor(ov, i0, i1, ALU.is_gt)
        rin = _vap(cmp_t[:], [[64, 2], [8, 8], [1, 8]])
        nc.vector.reduce_sum(cnt[:], rin, axis=AX.X)
        nc.vector.tensor_scalar(mS[:], cnt[:], 3.5, None, ALU.is_lt)
        nc.scalar.dma_start(mask_d[:], mS[:])

        # ---- per-block GRU with fp8 DoubleRow matmuls (pu = gi*2^14,
        # pvh = gh*2^14); transposes + qkv projection pipelined one block late.
        def emit_tq(bt, blk):
            tp2 = ps_sm.tile([128, 256], bf16, tag="sm")
            for ki in range(2):
                ft = blk * 2 + ki
                nc.tensor.transpose(
                    tp2[:, ki * 128:(ki + 1) * 128],
                    hP[:, bt * NHID + ft * 128: bt * NHID + (ft + 1) * 128],
                    identB[:])
            hdst = hT[:, blk * 2 * BL + bt * 128: blk * 2 * BL + bt * 128 + 1]
            nc.scalar.activation(_vap(hdst, [[BL, 2], [1, 128]]), tp2[:], AF.Copy)
            pqkv = ps_sm.tile([128, 192], f32, tag="sm")
            for ki in range(2):
                t_idx = blk * 2 + ki
                lhs = hT[:, t_idx * BL + bt * 128: t_idx * BL + (bt + 1) * 128]
                nc.tensor.matmul(pqkv[:], lhs,
                                 wqkv[:, t_idx * 192:(t_idx + 1) * 192],
                                 start=(ki == 0), stop=(ki == 1))
            qb = qk2P[:, bt * 1024 + blk * 64: bt * 1024 + blk * 64 + 1]
            nc.scalar.activation(_vap(qb, [[512, 2], [1, 64]]),
                                 pqkv[:, 0:128], AF.Copy)
            vsl = v2P[:, bt * 512 + blk: bt * 512 + blk + 1]
            nc.scalar.activation(_vap(vsl, [[128, 4], [8, 16]]),
                                 pqkv[:, 128:192], AF.Copy)

        def attn_stages(bt):
            # comm attention for one 128-row half, split into 4 dispensable
            # stages so they interleave with the other half's GRU emission.
            outS = p32.tile([128, NHID], f32, tag="big32")
            l0 = bt * 256
            s0 = bt * 32

            def s_pass1():
                with nc.allow_low_precision("bf16 comm-attn accumulation"):
                    pr = prodp.tile([128, 4096], bf16, tag="pr")
                    q0 = bt * 1024
                    i0 = _vap(qk2P[:, q0:q0 + 1],
                              [[64, 8], [0, 8], [16, 4], [1, 16]])
                    i1 = _vap(qk2P[:, bt * 1024 + 512: bt * 1024 + 513],
                              [[0, 8], [64, 8], [16, 4], [1, 16]])
                    ovp = _vap(pr[:], [[512, 8], [64, 8], [16, 4], [1, 16]])
                    nc.gpsimd.tensor_tensor(ovp, i0, i1, ALU.mult)
                    rin = _vap(pr[:], [[512, 8], [16, 4], [64, 8], [1, 16]])
                    lo = _vap(Lp[:, l0:l0 + 1], [[32, 8], [8, 4], [1, 8]])
                    nc.vector.reduce_sum(lo, rin, axis=AX.X)
                    esl = slice(l0, l0 + 256)
                    nc.scalar.activation(attE[:, esl], Lp[:, esl], AF.Exp)
                    sin = _vap(attE[:, l0:l0 + 1], [[32, 8], [8, 4], [1, 8]])
                    so = _vap(attS[:, s0:s0 + 1], [[4, 8], [1, 4]])
                    nc.vector.reduce_sum(so, sin, axis=AX.X)
                    nc.vector.reciprocal(attR[:, s0:s0 + 32], attS[:, s0:s0 + 32])
                    ev = _vap(attE[:, l0:l0 + 1], [[32, 8], [8, 4], [1, 8]])
                    rv = _vap(attR[:, s0:s0 + 1], [[4, 8], [1, 4], [0, 8]])
                    wv_o = _vap(attW[:, l0:l0 + 1], [[32, 8], [8, 4], [1, 8]])
                    nc.vector.tensor_tensor(wv_o, ev, rv, ALU.mult)

            def s_pass2():
                with nc.allow_low_precision("bf16 comm-attn accumulation"):
                    pv_ = prodp.tile([128, 4096], bf16, tag="pr")
                    av = _vap(attW[:, l0:l0 + 1],
                              [[32, 8], [8, 4], [0, 16], [1, 8]])
                    vv = _vap(v2P[:, bt * 512: bt * 512 + 1],
                              [[0, 8], [128, 4], [8, 16], [1, 8]])
                    pvv = _vap(pv_[:], [[512, 8], [128, 4], [8, 16], [1, 8]])
                    nc.gpsimd.tensor_tensor(pvv, av, vv, ALU.mult)
                    o0 = bt * 512
                    o2 = _vap(out2P[:, o0:o0 + 1], [[64, 8], [16, 4], [1, 16]])
                    nc.vector.reduce_sum(
                        o2, _vap(pv_[:], [[512, 8], [128, 4], [8, 16], [1, 8]]),
                        axis=AX.X)
                    for qp_i in range(4):
                        nc.sync.dma_start_transpose(
                            out2T[:, qp_i * 256 + bt * 128: qp_i * 256 + (bt + 1) * 128],
                            out2P[:, bt * 512 + qp_i * 128: bt * 512 + (qp_i + 1) * 128])

            def s_fg(g2a, g2b, dma):
                # fc|gate: block-diag fgw2 split into two 1-bank N=512 matmuls
                # (cols 0:512 hit only rows 0:64 = q-even, 512:1024 = q-odd)
                for g2 in range(g2a, g2b):
                    c0 = g2 * 256 + bt * 128
                    for j in range(2):  # q = 2*g2 + j
                        q = 2 * g2 + j
                        pfg = ps_fg.tile([128, 512], f32, tag="fg")
                        nc.tensor.matmul(pfg[:], out2T[:, c0:c0 + 128],
                                         fgw2[:, j * 512:(j + 1) * 512],
                                         start=True, stop=not has_gru_bias)
                        if has_gru_bias:
                            nc.tensor.matmul(pfg[:], onesR[:],
                                             fgb2[:, j * 512:(j + 1) * 512],
                                             start=False, stop=True)
                        gt = tailp.tile([128, BS], bf16, tag="gt")
                        ft_ = tailp.tile([128, BS], bf16, tag="ft")
                        nc.scalar.activation(gt[:], pfg[:, 256:512], AF.Sigmoid)
                        nc.scalar.activation(ft_[:], pfg[:, 0:256], AF.Tanh)
                        hatt = tailp.tile([128, BS], bf16, tag="hatt")
                        nc.gpsimd.tensor_tensor(hatt[:], gt[:], ft_[:], ALU.mult)
                        hx_sl = slice(bt * NHID + q * BS, bt * NHID + (q + 1) * BS)
                        d2 = tailp.tile([128, BS], bf16, tag="d2")
                        nc.gpsimd.tensor_tensor(d2[:], hd[:, hx_sl], hatt[:], ALU.add)
                        qcol = bt * 8 + q
                        nc.vector.scalar_tensor_tensor(
                            outS[:, q * BS:(q + 1) * BS], d2[:],
                            mS[:, qcol:qcol + 1], hxP_f[:, hx_sl],
                            ALU.mult, ALU.add)
                if dma:
                    nc.scalar.dma_start(out_d[bt][:], outS[:])

            return [s_pass1, s_pass2,
                    lambda: s_fg(0, 2, False), lambda: s_fg(2, 4, True)]

        pending = []   # attn stages of the previous bt, dispensed into this loop
        for bt in range(2):
            for blk in range(NB):
                col = bt * 8 + blk
                s_col = sS[:, col:col + 1]
                # gi: 4 DoubleRow steps over ki-pairs, two PSUM regions
                pu = ps_u.tile([128, G3], f32, tag="pu")
                for kk in range(4):
                    ki = 2 * kk
                    lhsT = _vap(inpT8[:, bt * 1024 + ki * 128: bt * 1024 + ki * 128 + 1],
                                [[128, 2], [1, 128]])
                    w0 = blk * 6144 + ki * G3
                    nc.tensor.matmul(pu[:, 0:512], lhsT,
                                     _vap(w28[:, w0:w0 + 1], [[G3, 2], [1, 512]]),
                                     start=(kk == 0), stop=(kk == 3),
                                     perf_mode=DR)
                    nc.tensor.matmul(pu[:, 512:G3], lhsT,
                                     _vap(w28[:, w0 + 512:w0 + 513], [[G3, 2], [1, 256]]),
                                     start=(kk == 0), stop=(kk == 3),
                                     perf_mode=DR)
                # gh: one DoubleRow step (contraction 256 = whole block)
                pvh = ps_v.tile([128, G3], f32, tag="pvh")
                hx0 = bt * 2048 + blk * 2 * 128
                lhsT = _vap(hxT8[:, hx0:hx0 + 1], [[128, 2], [1, 128]])
                wh0 = blk * 2 * G3
                nc.tensor.matmul(pvh[:, 0:512], lhsT,
                                 _vap(whh[:, wh0:wh0 + 1], [[G3, 2], [1, 512]]),
                                 start=True, stop=True, perf_mode=DR)
                nc.tensor.matmul(pvh[:, 512:G3], lhsT,
                                 _vap(whh[:, wh0 + 512:wh0 + 513], [[G3, 2], [1, 256]]),
                                 start=True, stop=True, perf_mode=DR)
                # gates: rz = sigmoid((s*gi_rz + gh_rz) * 2^-14).  STT cannot
                # read two PSUM operands, so gh_rz goes through SBUF first.
                pvc = gwork.tile([128, 512], bf16, tag="pvc")
                if blk % 2 == 0:
                    nc.vector.tensor_copy(pvc[:], pvh[:, 0:512])
                else:
                    nc.scalar.activation(pvc[:], pvh[:, 0:512], AF.Copy)
                rzp = gwork.tile([128, 512], f32, tag="rzp")
                nc.vector.scalar_tensor_tensor(
                    rzp[:], pu[:, 0:512], s_col, pvc[:], ALU.mult, ALU.add)
                if has_gru_bias:
                    nc.vector.tensor_tensor(rzp[:], rzp[:],
                                            bbB[:, blk * G3: blk * G3 + 512], ALU.add)
                rzs = gwork.tile([128, 512], bf16, tag="rzs")
                nc.scalar.activation(rzs[:], rzp[:], AF.Sigmoid, scale=PSC)
                rhn = gwork.tile([128, BS], f32, tag="rhn")
                nc.vector.tensor_tensor(rhn[:], rzs[:, 0:BS], pvh[:, 512:G3], ALU.mult)
                npre = gwork.tile([128, BS], f32, tag="npre")
                nc.vector.scalar_tensor_tensor(
                    npre[:], pu[:, 512:G3], s_col, rhn[:], ALU.mult, ALU.add)
                if has_gru_bias:
                    nc.vector.tensor_tensor(
                        npre[:], npre[:],
                        bbB[:, blk * G3 + 512: (blk + 1) * G3], ALU.add)
                nt = gwork.tile([128, BS], bf16, tag="nt")
                nc.scalar.activation(nt[:], npre[:], AF.Tanh, scale=PSC)
                # h' = n + z*(h-n);  hd = h' - h = (z-1)*(n-h)... = zd - dt
                hsl = slice(bt * NHID + blk * BS, bt * NHID + (blk + 1) * BS)
                dt_ = gwork.tile([128, BS], bf16, tag="dt")
                nc.gpsimd.tensor_tensor(dt_[:], hxP_f[:, hsl], nt[:], ALU.subtract)
                zd = gwork.tile([128, BS], bf16, tag="zd")
                nc.gpsimd.tensor_tensor(zd[:], rzs[:, BS:512], dt_[:], ALU.mult)
                nc.gpsimd.tensor_tensor(hP[:, hsl], nt[:], zd[:], ALU.add)
                nc.vector.tensor_tensor(hd[:, hsl], zd[:], dt_[:], ALU.subtract)
                if blk > 0:
                    emit_tq(bt, blk - 1)
                if pending and blk >= 2 and blk % 2 == 0:
                    pending.pop(0)()
            emit_tq(bt, NB - 1)
            if bt == 0:
                pending = attn_stages(0)
            else:
                while pending:
                    pending.pop(0)()
                for st in attn_stages(1):
                    st()

    nc.compile()
    return nc


_CACHE = {}


def _get_nc(has_gru_bias: bool):
    if has_gru_bias not in _CACHE:
        _CACHE[has_gru_bias] = _build(has_gru_bias)
    return _CACHE[has_gru_bias]


def _q8(x, scale):
    y = np.clip(np.asarray(x, np.float32) * scale, -240.0, 240.0)
    return np.ascontiguousarray(y).astype(F8)


def _prep(inputs):
    """Host-side sharding / layout prep. Returns (in_maps, has_gru_bias)."""
    inp = np.asarray(inputs["inp"], np.float32)
    hx = np.asarray(inputs["hx"], np.float32)
    has_gru_bias = bool(
        np.any(np.asarray(inputs["bih"])) or np.any(np.asarray(inputs["bhh"]))
        or np.any(np.asarray(inputs["fc_b"])) or np.any(np.asarray(inputs["gate_b"])))

    # ---- shared weight layouts (same for every core)
    Wk1 = np.asarray(inputs["Wk1"], np.float32)[1]            # (1024, 64)
    wk1 = Wk1.reshape(KI_IN, 128, 64).transpose(1, 0, 2).reshape(128, KI_IN * 64)
    wk1 = np.ascontiguousarray(wk1, np.float32)
    Wq1 = np.asarray(inputs["Wq1"], np.float32)               # (8, 256, 64)
    wq1 = np.ascontiguousarray(
        Wq1.reshape(NB, 2, 128, 64).transpose(2, 0, 1, 3)
        .reshape(128, KI_HID * 64), np.float32)
    # W2[k] = Wv1[1] @ Wih[k]^T  (1024, 768) folded on host
    Wv1 = np.asarray(inputs["Wv1"], np.float32)[1]            # (1024, 1024)
    Wih = np.asarray(inputs["Wih"], np.float32)               # (8, 768, 1024)
    W2 = np.matmul(Wv1[None], Wih.transpose(0, 2, 1))         # (8, 1024, 768)
    w28 = _q8(
        W2.reshape(NB, KI_IN, 128, G3).transpose(2, 0, 1, 3)
        .reshape(128, NB * KI_IN * G3), S_W2)
    Whh = np.asarray(inputs["Whh"], np.float32)               # (8, 768, 256)
    whh = _q8(
        Whh.transpose(0, 2, 1).reshape(NB, 2, 128, G3)
        .transpose(2, 0, 1, 3).reshape(128, NB * 2 * G3), S_WHH)

    def proj_layout(w, scale=1.0):
        t = (np.asarray(w, np.float32) * scale).reshape(NB, 2, 128, 64)
        return np.ascontiguousarray(t.transpose(2, 0, 1, 3)
                                    .reshape(128, NB * 2, 64))

    wqkv = np.concatenate([proj_layout(inputs["Wq2"]),
                           proj_layout(inputs["Wk2"], 0.25),   # 1/sqrt(DK2)
                           proj_layout(inputs["Wv2"])], axis=2)
    wqkv = np.ascontiguousarray(wqkv.reshape(128, NB * 2 * 192)).astype(BF)
    fg = np.concatenate([np.asarray(inputs["fc_w"], np.float32),
                         np.asarray(inputs["gate_w"], np.float32)], axis=1)  # (64,512)
    fgw2 = np.zeros((128, 1024), np.float32)
    fgw2[0:64, 0:512] = fg
    fgw2[64:128, 512:1024] = fg
    fgw2 = fgw2.astype(BF)

    shared = dict(wk1=wk1, wq1=wq1, w28=w28, whh=whh, wqkv=wqkv, fgw2=fgw2)
    if has_gru_bias:
        bb = (np.asarray(inputs["bih"], np.float32)
              + np.asarray(inputs["bhh"], np.float32)) * (2.0 ** 14)  # (8,768)
        shared["bbB"] = np.ascontiguousarray(
            np.broadcast_to(bb.reshape(1, NB * G3), (128, NB * G3)), np.float32)
        shared["onesrow"] = np.ones((1, 128), BF)
        fgb = np.concatenate([np.asarray(inputs["fc_b"], np.float32),
                              np.asarray(inputs["gate_b"], np.float32)])
        shared["fgb2"] = np.concatenate([fgb, fgb]).reshape(1, 1024).astype(BF)

    in_maps = []
    for c in range(NCORES):
        r0 = c * BL
        inp_s = inp[r0:r0 + BL]                               # (256, 1024)
        hx_s = hx[r0:r0 + BL]                                 # (256, 2048)
        inpT = np.ascontiguousarray(
            inp_s.T.reshape(KI_IN, 128, BL).transpose(1, 0, 2)
            .reshape(128, KI_IN * BL), np.float32)
        # [p, bt*1024 + ki*128 + c] layouts for fp8/fp32 transposed tensors
        inpT8 = _q8(
            inp_s.T.reshape(KI_IN, 128, 2, 128).transpose(1, 2, 0, 3)
            .reshape(128, 2 * KI_IN * 128), S_INP)
        hxT4 = hx_s.T.reshape(KI_HID, 128, 2, 128).transpose(1, 2, 0, 3) \
            .reshape(128, 2 * KI_HID * 128)
        hxT8 = _q8(hxT4, S_INP)
        hxTf = np.ascontiguousarray(hxT4, np.float32)
        hxP = np.ascontiguousarray(
            hx_s.reshape(2, 128, NHID).transpose(1, 0, 2)
            .reshape(128, 2 * NHID), np.float32)
        m = dict(inpT=inpT, inpT8=inpT8, hxT8=hxT8, hxTf=hxTf, hxP=hxP, **shared)
        in_maps.append(m)
    return in_maps, has_gru_bias


_EXEC = {}


def _get_exec(nc, key):
    """Build (once) a cached jitted SPMD executor for `nc` (axon/PJRT path)."""
    if key in _EXEC:
        return _EXEC[key]
    import jax
    from jax.sharding import Mesh, PartitionSpec
    from jax.experimental.shard_map import shard_map
    from concourse import bass2jax
    from concourse.bass2jax import _bass_exec_p

    bass2jax.install_neuronx_cc_hook()

    partition_name = (nc.partition_id_tensor.name
                      if nc.partition_id_tensor else None)
    in_names, out_names, out_avals, zero_shapes = [], [], [], []
    for alloc in nc.m.functions[0].allocations:
        if not isinstance(alloc, mybir.MemoryLocationSet):
            continue
        name = alloc.memorylocations[0].name
        if alloc.kind == "ExternalInput":
            if name != partition_name:
                in_names.append(name)
        elif alloc.kind == "ExternalOutput":
            out_names.append(name)
            shape = tuple(alloc.tensor_shape)
            dtype = mybir.dt.np(alloc.dtype)
            out_avals.append(jax.core.ShapedArray(shape, dtype))
            zero_shapes.append((shape, dtype))
    n_params = len(in_names)
    all_names = list(in_names) + list(out_names)
    if partition_name is not None:
        all_names.append(partition_name)

    def _body(*args):
        operands = list(args)
        if partition_name is not None:
            operands.append(bass2jax.partition_id_tensor())
        outs = _bass_exec_p.bind(
            *operands,
            out_avals=tuple(out_avals),
            in_names=tuple(all_names),
            out_names=tuple(out_names),
            lowering_input_output_aliases=(),
            sim_require_finite=True,
            sim_require_nnan=True,
            nc=nc,
        )
        return tuple(outs)

    donate = tuple(range(n_params, n_params + len(out_names)))
    devices = jax.devices()[:NCORES]
    mesh = Mesh(np.asarray(devices), ("core",))
    in_specs = (PartitionSpec("core"),) * (n_params + len(out_names))
    out_specs = (PartitionSpec("core"),) * len(out_names)
    sharded = jax.jit(
        shard_map(_body, mesh=mesh, in_specs=in_specs, out_specs=out_specs,
                  check_rep=False),
        donate_argnums=donate, keep_unused=True)

    _EXEC[key] = (sharded, in_names, out_names, zero_shapes)
    return _EXEC[key]


def run_prepared(in_maps, has_gru_bias, iters=1):
    """Execute the compiled kernel on 8 cores; returns (per-core out arrays,
    list of per-iteration wall seconds)."""
    import time
    import jax
    from jax.sharding import NamedSharding, PartitionSpec
    nc = _get_nc(has_gru_bias)
    sharded, in_names, out_names, zero_shapes = _get_exec(nc, has_gru_bias)
    concat_in = [np.concatenate([np.asarray(m[n]) for m in in_maps], axis=0)
                 for n in in_names]
    times = []
    if iters > 1:
        from jax.sharding import Mesh
        mesh = Mesh(np.asarray(jax.devices()[:NCORES]), ("core",))
        sh = NamedSharding(mesh, PartitionSpec("core"))
        concat_in = [jax.device_put(a, sh) for a in concat_in]
        zero_sets = []
        for _ in range(iters):
            zero_sets.append([
                jax.device_put(np.zeros((NCORES * s[0], *s[1:]), d), sh)
                for s, d in zero_shapes])
        jax.block_until_ready(concat_in)
        jax.block_until_ready(zero_sets)
        out_arrs = sharded(*concat_in, *zero_sets[0])
        jax.block_until_ready(out_arrs)
        t0 = time.perf_counter()
        for i in range(1, iters):
            out_arrs = sharded(*concat_in, *zero_sets[i])
        jax.block_until_ready(out_arrs)
        dt = (time.perf_counter() - t0) / (iters - 1)
        times = [dt] * iters
        out_arrs = [np.asarray(a) for a in out_arrs]
    else:
        zeros = [np.zeros((NCORES * s[0], *s[1:]), d) for s, d in zero_shapes]
        t0 = time.perf_counter()
        out_arrs = sharded(*concat_in, *zeros)
        jax.block_until_ready(out_arrs)
        out_arrs = [np.asarray(a) for a in out_arrs]
        times.append(time.perf_counter() - t0)
    i = out_names.index("out")
    j = out_names.index("maskout")
    full = out_arrs[i].reshape(NCORES, 2, 128, NHID)
    mfull = out_arrs[j].reshape(NCORES, 128, 16)
    return (full, mfull), times


def kernel(**inputs):
    in_maps, has_gru_bias = _prep(inputs)
    (full, mfull), _ = run_prepared(in_maps, has_gru_bias, iters=1)
    res = np.empty((B, NHID), np.float32)
    mask_blk = np.empty((B, NB), np.float32)
    for c in range(NCORES):
        res[c * BL:(c + 1) * BL] = full[c].reshape(BL, NHID)
        for bt in range(2):
            mask_blk[c * BL + bt * 128: c * BL + (bt + 1) * 128] = \
                mfull[c][:, bt * 8:(bt + 1) * 8]
    mask = np.repeat(mask_blk, BS, axis=1)
    return res, mask


# revision 33
# speedup vs baseline: 1.0757x; 1.0757x over previous
"""Trainium2 Bass kernel for nn_BlocksCore (RIMs-style BlocksCore forward).

Sharding: data-parallel over batch B=2048 across 8 NeuronCores (256 rows each,
zero cross-core communication; all model ops are batch-independent).

Key optimizations over the bf16 baseline:
  - W2 = Wv1[1] @ Wih^T is folded on the host, so the GRU input projection
    gi = s * (inp @ W2) consumes inp directly: the whole v1 matmul stage
    disappears and one fp8 quantization step is avoided.
  - gi and gh run as fp8e4 DoubleRow matmuls (2 k-subtiles per pass). Both
    arrive in PSUM scaled by 2^14 (inp*16 x W2*4096 / 16, hx*16 x Whh*1024),
    so s*gi+gh combines with one STT and the 2^-14 descale rides the
    sigmoid/tanh activation scale for free.
  - Routing stays fp32 (top-k margins are ~1e-6) but restructured: k1 and
    q = hx@Wq1 are computed row-major on the PE (half the fp32 columns of
    the old k1T/wp form) and the 64-dim dot runs on DVE.
  - bt-major pipeline: all 8 blocks of batch-half 0 run first, then its
    comm-attention chain (Vector-heavy) is emitted so it overlaps with
    batch-half 1's GRU matmuls on the PE.
  - fc and gate projections share one block-diagonal stationary matrix
    (fgw2), so each q-pair needs a single K=128 matmul instead of two
    half-wasted K=64 matmuls.
  - Elementwise work is spread V/S/G: STT gate-combines and small TTs on
    Vector, sigmoid/tanh/copies on Scalar, h' blend products on GpSimd.
"""
import sys

sys.path.insert(0, "/opt/trn_rl_repo")

import numpy as np
import ml_dtypes

import concourse.bass as bass
import concourse.tile as tile
from concourse.masks import make_identity
from concourse import bacc, mybir

f32 = mybir.dt.float32
bf16 = mybir.dt.bfloat16
fp8 = mybir.dt.float8e4
AF = mybir.ActivationFunctionType
ALU = mybir.AluOpType
AX = mybir.AxisListType
DR = mybir.MatmulPerfMode.DoubleRow

B, NINP, NHID = 2048, 1024, 2048
NB, BS, G3 = 8, 256, 768          # blocks, block_size_out, 3*BS
NH2, DK2, DV2, HD = 4, 16, 16, 64  # comm attn heads, dims, NH2*DV2
NCORES = 8
BL = B // NCORES                   # 256 rows per core
KI_IN = NINP // 128                # 8
KI_HID = NHID // 128               # 16

BF = ml_dtypes.bfloat16
F8 = ml_dtypes.float8_e4m3

# power-of-two quantization scales (values clipped to +-240 on host)
S_INP = 16.0      # inp, hx fp8 copies
S_W2 = 1024.0     # W2 = Wv1 @ Wih^T
S_WHH = 1024.0
# pu = (inp*16) @ (W2*1024) = gi * 2^14 ; pvh = (hx*16) @ (Whh*1024) = gh * 2^14
PSC = 2.0 ** -14  # descale applied inside sigmoid/tanh activation scale


def _vap(sl, dims):
    """Custom free-dim view: keep partition dim of slice `sl`, replace free dims."""
    return bass.AP(sl.tensor, sl.offset, [sl.ap[0]] + [list(d) for d in dims])


def _build(has_gru_bias: bool):
    nc = bacc.Bacc("TRN2", target_bir_lowering=False, debug=False,
                   num_devices=NCORES)

    def din(name, shape, dt):
        return nc.dram_tensor(name, list(shape), dt, kind="ExternalInput").ap()

    inpT_d = din("inpT", (128, KI_IN * BL), f32)          # [p, ki*256+c]
    inpT8_d = din("inpT8", (128, 2 * KI_IN * 128), fp8)   # [p, bt*1024+ki*128+c]
    hxP_d = din("hxP", (128, 2 * NHID), f32)              # [p, bt*2048+f]
    hxT8_d = din("hxT8", (128, 2 * KI_HID * 128), fp8)    # [p, bt*2048+t*128+c]
    wk1_d = din("wk1", (128, KI_IN * 64), f32)            # [p, ki*64+d]
    wq1t_d = din("wq1t", (64, NB * BS), f32)              # [d, blk*256+f]
    w28_d = din("w28", (128, NB * KI_IN * G3), fp8)       # [p, blk*6144+ki*768+g]
    whh_d = din("whh", (128, NB * 2 * G3), fp8)           # [p, blk*1536+ki*768+g]
    wqkv_d = din("wqkv", (128, NB * 2 * 192), bf16)       # [p, t*192 + {q2|k2/4|v2}]
    fgw2_d = din("fgw2", (128, 1024), bf16)               # block-diag [fc|gate; fc|gate]
    if has_gru_bias:
        bbB_d = din("bbB", (128, NB * G3), f32)           # (bih+bhh)*2^14 bcast
        ones_d = din("onesrow", (1, 128), bf16)
        fgb2_d = din("fgb2", (1, 1024), bf16)             # fc_b|gate_b twice
    out_d = nc.dram_tensor("out", [2, 128, NHID], f32, kind="ExternalOutput").ap()
    mask_d = nc.dram_tensor("maskout", [128, 16], f32, kind="ExternalOutput").ap()

    from contextlib import ExitStack
    with tile.TileContext(nc) as tc, ExitStack() as ctx:
        P = ctx.enter_context(tc.tile_pool(name="persist", bufs=1))
        p32 = ctx.enter_context(tc.tile_pool(name="p32", bufs=2))
        gwork = ctx.enter_context(tc.tile_pool(name="gwork", bufs=2))
        prodp = ctx.enter_context(tc.tile_pool(name="prodp", bufs=1))
        tailp = ctx.enter_context(tc.tile_pool(name="tailp", bufs=2))
        # PSUM budget, bank-granular (8x2KB): pu 2x2 + pvh 1x2 + sm 1x1 + fg 1x1
        ps_u = ctx.enter_context(tc.tile_pool(name="ps_u", bufs=2, space="PSUM"))
        ps_v = ctx.enter_context(tc.tile_pool(name="ps_v", bufs=1, space="PSUM"))
        ps_sm = ctx.enter_context(tc.tile_pool(name="ps_sm", bufs=1, space="PSUM"))
        ps_fg = ctx.enter_context(tc.tile_pool(name="ps_fg", bufs=1, space="PSUM"))

        # ---- persistent sbuf tensors
        inpT8 = P.tile([128, 2 * KI_IN * 128], fp8, tag="inpT8")
        hxP_f = P.tile([128, 2 * NHID], f32, tag="hxP_f")
        hxT8 = P.tile([128, 2 * KI_HID * 128], fp8, tag="hxT8")
        wk1 = P.tile([128, KI_IN * 64], f32, tag="wk1")
        wq1t = P.tile([64, NB * BS], f32, tag="wq1t")
        w28 = P.tile([128, NB * KI_IN * G3], fp8, tag="w28")
        whh = P.tile([128, NB * 2 * G3], fp8, tag="whh")
        wqkv = P.tile([128, NB * 2 * 192], bf16, tag="wqkv")
        fgw2 = P.tile([128, 1024], bf16, tag="fgw2")
        k1T = P.tile([64, BL], f32, tag="k1T")
        aP = P.tile([128, 16], f32, tag="aP")
        sS = P.tile([128, 16], f32, tag="sS")
        mS = P.tile([128, 16], f32, tag="mS")
        cnt = P.tile([128, 16], f32, tag="cnt")
        cmp_t = P.tile([128, 128], f32, tag="cmp")
        hP = P.tile([128, 2 * NHID], bf16, tag="hP")
        hd = P.tile([128, 2 * NHID], bf16, tag="hd")
        hT = P.tile([128, KI_HID * BL], bf16, tag="hT")
        qk2P = P.tile([128, 2 * 1024], bf16, tag="qk2P")   # [bt*1024 + {0:q2,512:k2} + blk*64]
        v2P = P.tile([128, 2 * NB * 64], bf16, tag="v2P")  # [bt*512 + h*128 + d*8 + k]
        Lp = P.tile([128, 2 * 256], bf16, tag="Lp")
        attE = P.tile([128, 2 * 256], bf16, tag="attE")
        attS = P.tile([128, 2 * 32], bf16, tag="attS")
        attR = P.tile([128, 2 * 32], bf16, tag="attR")
        attW = P.tile([128, 2 * 256], bf16, tag="attW")
        out2P = P.tile([128, 2 * NB * 64], bf16, tag="out2P")
        out2T = P.tile([128, 4 * BL], bf16, tag="out2T")  # [(q%2)*64+hd, (q//2)*256+bt*128+b]
        identB = P.tile([128, 128], bf16, tag="identB")
        if has_gru_bias:
            bbB = P.tile([128, NB * G3], f32, tag="bbB")
            onesR = P.tile([1, 128], bf16, tag="onesR")
            fgb2 = P.tile([1, 1024], bf16, tag="fgb2")

        inpT_f = p32.tile([128, KI_IN * BL], f32, tag="big32")

        # ---- DMA emission. sync queue: routing-critical inp stream + fp8 GRU
        # weights; scalar queue: hx stream + attn weights.
        nc.sync.dma_start(wk1[:], wk1_d[:])
        nc.sync.dma_start(inpT_f[:], inpT_d[:])
        nc.sync.dma_start(inpT8[:], inpT8_d[:])
        nc.sync.dma_start(whh[:], whh_d[:])
        for blk in range(NB):
            nc.sync.dma_start(w28[:, blk * 6144:(blk + 1) * 6144],
                              w28_d[:, blk * 6144:(blk + 1) * 6144])
        nc.scalar.dma_start(wq1t[:], wq1t_d[:])
        nc.scalar.dma_start(hxP_f[:, 0:NHID], hxP_d[:, 0:NHID])
        nc.scalar.dma_start(hxT8[:], hxT8_d[:])
        nc.scalar.dma_start(hxP_f[:, NHID:], hxP_d[:, NHID:])
        nc.scalar.dma_start(wqkv[:], wqkv_d[:])
        nc.scalar.dma_start(fgw2[:], fgw2_d[:])
        if has_gru_bias:
            nc.scalar.dma_start(bbB[:], bbB_d[:])
            nc.scalar.dma_start(onesR[:], ones_d[:])
            nc.scalar.dma_start(fgb2[:], fgb2_d[:])
        make_identity(nc, identB[:])

        # ---- routing (all fp32, wide-N matmuls): k1T = (inp@Wk1)^T, then
        # a[row,blk] = sum_f (k1T^T @ Wq1^T)[row,f] * hx[row,f] via STT accum.
        kp = ps_sm.tile([64, BL], f32, tag="sm")
        for ki in range(KI_IN):
            nc.tensor.matmul(kp[:], wk1[:, ki * 64:(ki + 1) * 64],
                             inpT_f[:, ki * BL:(ki + 1) * BL],
                             start=(ki == 0), stop=(ki == KI_IN - 1))
        nc.vector.tensor_copy(k1T[:], kp[:])
        for bt in range(2):
            for blk in range(NB):
                wp = ps_fg.tile([128, BS], f32, tag="fg")
                nc.tensor.matmul(wp[:], k1T[:, bt * 128:(bt + 1) * 128],
                                 wq1t[:, blk * BS:(blk + 1) * BS],
                                 start=True, stop=True)
                scr = gwork.tile([128, BS], f32, tag="scr")
                col = bt * 8 + blk
                nc.vector.scalar_tensor_tensor(
                    scr[:], wp[:], 0.125,
                    hxP_f[:, bt * NHID + blk * BS: bt * NHID + (blk + 1) * BS],
                    ALU.mult, ALU.mult, accum_out=aP[:, col:col + 1])
        nc.scalar.activation(sS[:], aP[:], AF.Sigmoid)
        # mask: cnt[bt,k] = #{j : a[bt,j] > a[bt,k]};  keep iff cnt < 4
        i0 = _vap(aP[:], [[8, 2], [1, 8], [0, 8]])
        i1 = _vap(aP[:], [[8, 2], [0, 8], [1, 8]])
        ov = _vap(cmp_t[:], [[64, 2], [1, 8], [8, 8]])
        nc.vector.tensor_tensor(ov, i0, i1, ALU.is_gt)
        rin = _vap(cmp_t[:], [[64, 2], [8, 8], [1, 8]])
        nc.vector.reduce_sum(cnt[:], rin, axis=AX.X)
        nc.vector.tensor_scalar(mS[:], cnt[:], 3.5, None, ALU.is_lt)
        nc.scalar.dma_start(mask_d[:], mS[:])

        # ---- per-block GRU with fp8 DoubleRow matmuls (pu = gi*2^14,
        # pvh = gh*2^14); transposes + qkv projection pipelined one block late.
        def emit_tq(bt, blk):
            tp2 = ps_sm.tile([128, 256], bf16, tag="sm")
            for ki in range(2):
                ft = blk * 2 + ki
                nc.tensor.transpose(
                    tp2[:, ki * 128:(ki + 1) * 128],
                    hP[:, bt * NHID + ft * 128: bt * NHID + (ft + 1) * 128],
                    identB[:])
            hdst = hT[:, blk * 2 * BL + bt * 128: blk * 2 * BL + bt * 128 + 1]
            nc.scalar.activation(_vap(hdst, [[BL, 2], [1, 128]]), tp2[:], AF.Copy)
            pqkv = ps_sm.tile([128, 192], f32, tag="sm")
            for ki in range(2):
                t_idx = blk * 2 + ki
                lhs = hT[:, t_idx * BL + bt * 128: t_idx * BL + (bt + 1) * 128]
                nc.tensor.matmul(pqkv[:], lhs,
                                 wqkv[:, t_idx * 192:(t_idx + 1) * 192],
                                 start=(ki == 0), stop=(ki == 1))
            qb = qk2P[:, bt * 1024 + blk * 64: bt * 1024 + blk * 64 + 1]
            nc.scalar.activation(_vap(qb, [[512, 2], [1, 64]]),
                                 pqkv[:, 0:128], AF.Copy)
            vsl = v2P[:, bt * 512 + blk: bt * 512 + blk + 1]
            nc.scalar.activation(_vap(vsl, [[128, 4], [8, 16]]),
                                 pqkv[:, 128:192], AF.Copy)

        def attn_stages(bt):
            # comm attention for one 128-row half, split into 4 dispensable
            # stages so they interleave with the other half's GRU emission.
            outS = p32.tile([128, NHID], f32, tag="big32")
            l0 = bt * 256
            s0 = bt * 32

            def s_pass1():
                with nc.allow_low_precision("bf16 comm-attn accumulation"):
                    pr = prodp.tile([128, 4096], bf16, tag="pr")
                    q0 = bt * 1024
                    i0 = _vap(qk2P[:, q0:q0 + 1],
                              [[64, 8], [0, 8], [16, 4], [1, 16]])
                    i1 = _vap(qk2P[:, bt * 1024 + 512: bt * 1024 + 513],
                              [[0, 8], [64, 8], [16, 4], [1, 16]])
                    ovp = _vap(pr[:], [[512, 8], [64, 8], [16, 4], [1, 16]])
                    nc.vector.tensor_tensor(ovp, i0, i1, ALU.mult)
                    rin = _vap(pr[:], [[512, 8], [16, 4], [64, 8], [1, 16]])
                    lo = _vap(Lp[:, l0:l0 + 1], [[32, 8], [8, 4], [1, 8]])
                    nc.vector.reduce_sum(lo, rin, axis=AX.X)
                    esl = slice(l0, l0 + 256)
                    nc.scalar.activation(attE[:, esl], Lp[:, esl], AF.Exp)
                    sin = _vap(attE[:, l0:l0 + 1], [[32, 8], [8, 4], [1, 8]])
                    so = _vap(attS[:, s0:s0 + 1], [[4, 8], [1, 4]])
                    nc.vector.reduce_sum(so, sin, axis=AX.X)
                    nc.vector.reciprocal(attR[:, s0:s0 + 32], attS[:, s0:s0 + 32])
                    ev = _vap(attE[:, l0:l0 + 1], [[32, 8], [8, 4], [1, 8]])
                    rv = _vap(attR[:, s0:s0 + 1], [[4, 8], [1, 4], [0, 8]])
                    wv_o = _vap(attW[:, l0:l0 + 1], [[32, 8], [8, 4], [1, 8]])
                    nc.vector.tensor_tensor(wv_o, ev, rv, ALU.mult)

            def s_pass2():
                with nc.allow_low_precision("bf16 comm-attn accumulation"):
                    pv_ = prodp.tile([128, 4096], bf16, tag="pr")
                    av = _vap(attW[:, l0:l0 + 1],
                              [[32, 8], [8, 4], [0, 16], [1, 8]])
                    vv = _vap(v2P[:, bt * 512: bt * 512 + 1],
                              [[0, 8], [128, 4], [8, 16], [1, 8]])
                    pvv = _vap(pv_[:], [[512, 8], [128, 4], [8, 16], [1, 8]])
                    nc.vector.tensor_tensor(pvv, av, vv, ALU.mult)
                    o0 = bt * 512
                    o2 = _vap(out2P[:, o0:o0 + 1], [[64, 8], [16, 4], [1, 16]])
                    nc.vector.reduce_sum(
                        o2, _vap(pv_[:], [[512, 8], [128, 4], [8, 16], [1, 8]]),
                        axis=AX.X)
                    for qp_i in range(4):
                        eng = nc.sync if qp_i % 2 == 0 else nc.scalar
                        eng.dma_start_transpose(
                            out2T[:, qp_i * 256 + bt * 128: qp_i * 256 + (bt + 1) * 128],
                            out2P[:, bt * 512 + qp_i * 128: bt * 512 + (qp_i + 1) * 128])

            def s_fg(g2a, g2b, dma):
                # fc|gate: block-diag fgw2 split into two 1-bank N=512 matmuls
                # (cols 0:512 hit only rows 0:64 = q-even, 512:1024 = q-odd)
                for g2 in range(g2a, g2b):
                    c0 = g2 * 256 + bt * 128
                    for j in range(2):  # q = 2*g2 + j
                        q = 2 * g2 + j
                        pfg = ps_fg.tile([128, 512], f32, tag="fg")
                        nc.tensor.matmul(pfg[:], out2T[:, c0:c0 + 128],
                                         fgw2[:, j * 512:(j + 1) * 512],
                                         start=True, stop=not has_gru_bias)
                        if has_gru_bias:
                            nc.tensor.matmul(pfg[:], onesR[:],
                                             fgb2[:, j * 512:(j + 1) * 512],
                                             start=False, stop=True)
                        gt = tailp.tile([128, BS], bf16, tag="gt")
                        ft_ = tailp.tile([128, BS], bf16, tag="ft")
                        nc.scalar.activation(gt[:], pfg[:, 256:512], AF.Sigmoid)
                        nc.scalar.activation(ft_[:], pfg[:, 0:256], AF.Tanh)
                        hatt = tailp.tile([128, BS], bf16, tag="hatt")
                        nc.gpsimd.tensor_tensor(hatt[:], gt[:], ft_[:], ALU.mult)
                        hx_sl = slice(bt * NHID + q * BS, bt * NHID + (q + 1) * BS)
                        d2 = tailp.tile([128, BS], bf16, tag="d2")
                        nc.gpsimd.tensor_tensor(d2[:], hd[:, hx_sl], hatt[:], ALU.add)
                        qcol = bt * 8 + q
                        nc.vector.scalar_tensor_tensor(
                            outS[:, q * BS:(q + 1) * BS], d2[:],
                            mS[:, qcol:qcol + 1], hxP_f[:, hx_sl],
                            ALU.mult, ALU.add)
                if g2b == 2:
                    nc.scalar.dma_start(out_d[bt][:, 0:1024], outS[:, 0:1024])
                if dma:
                    nc.scalar.dma_start(out_d[bt][:, 1024:2048], outS[:, 1024:2048])

            return [s_pass1, s_pass2,
                    lambda: s_fg(0, 2, False), lambda: s_fg(2, 4, True)]

        pending = []   # attn stages of the previous bt, dispensed into this loop
        for bt in range(2):
            for blk in range(NB):
                col = bt * 8 + blk
                s_col = sS[:, col:col + 1]
                # gi: 4 DoubleRow steps over ki-pairs, two PSUM regions
                pu = ps_u.tile([128, G3], f32, tag="pu")
                for kk in range(4):
                    ki = 2 * kk
                    lhsT = _vap(inpT8[:, bt * 1024 + ki * 128: bt * 1024 + ki * 128 + 1],
                                [[128, 2], [1, 128]])
                    w0 = blk * 6144 + ki * G3
                    nc.tensor.matmul(pu[:, 0:512], lhsT,
                                     _vap(w28[:, w0:w0 + 1], [[G3, 2], [1, 512]]),
                                     start=(kk == 0), stop=(kk == 3),
                                     perf_mode=DR)
                    nc.tensor.matmul(pu[:, 512:G3], lhsT,
                                     _vap(w28[:, w0 + 512:w0 + 513], [[G3, 2], [1, 256]]),
                                     start=(kk == 0), stop=(kk == 3),
                                     perf_mode=DR)
                # gh: one DoubleRow step (contraction 256 = whole block)
                pvh = ps_v.tile([128, G3], f32, tag="pvh")
                hx0 = bt * 2048 + blk * 2 * 128
                lhsT = _vap(hxT8[:, hx0:hx0 + 1], [[128, 2], [1, 128]])
                wh0 = blk * 2 * G3
                nc.tensor.matmul(pvh[:, 0:512], lhsT,
                                 _vap(whh[:, wh0:wh0 + 1], [[G3, 2], [1, 512]]),
                                 start=True, stop=True, perf_mode=DR)
                nc.tensor.matmul(pvh[:, 512:G3], lhsT,
                                 _vap(whh[:, wh0 + 512:wh0 + 513], [[G3, 2], [1, 256]]),
                                 start=True, stop=True, perf_mode=DR)
                # gates: rz = sigmoid((s*gi_rz + gh_rz) * 2^-14).  STT cannot
                # read two PSUM operands, so gh_rz goes through SBUF first.
                pvc = gwork.tile([128, 512], bf16, tag="pvc")
                nc.scalar.activation(pvc[:], pvh[:, 0:512], AF.Copy)
                rzp = gwork.tile([128, 512], f32, tag="rzp")
                nc.vector.scalar_tensor_tensor(
                    rzp[:], pu[:, 0:512], s_col, pvc[:], ALU.mult, ALU.add)
                if has_gru_bias:
                    nc.vector.tensor_tensor(rzp[:], rzp[:],
                                            bbB[:, blk * G3: blk * G3 + 512], ALU.add)
                rzs = gwork.tile([128, 512], bf16, tag="rzs")
                nc.scalar.activation(rzs[:], rzp[:], AF.Sigmoid, scale=PSC)
                rhn = gwork.tile([128, BS], f32, tag="rhn")
                nc.vector.tensor_tensor(rhn[:], rzs[:, 0:BS], pvh[:, 512:G3], ALU.mult)
                npre = gwork.tile([128, BS], f32, tag="npre")
                nc.vector.scalar_tensor_tensor(
                    npre[:], pu[:, 512:G3], s_col, rhn[:], ALU.mult, ALU.add)
                if has_gru_bias:
                    nc.vector.tensor_tensor(
                        npre[:], npre[:],
                        bbB[:, blk * G3 + 512: (blk + 1) * G3], ALU.add)
                nt = gwork.tile([128, BS], bf16, tag="nt")
                nc.scalar.activation(nt[:], npre[:], AF.Tanh, scale=PSC)
                # h' = n + z*(h-n);  hd = h' - h = (z-1)*(n-h)... = zd - dt
                hsl = slice(bt * NHID + blk * BS, bt * NHID + (blk + 1) * BS)
                dt_ = gwork.tile([128, BS], bf16, tag="dt")
                nc.gpsimd.tensor_tensor(dt_[:], hxP_f[:, hsl], nt[:], ALU.subtract)
                zd = gwork.tile([128, BS], bf16, tag="zd")
                nc.gpsimd.tensor_tensor(zd[:], rzs[:, BS:512], dt_[:], ALU.mult)
                nc.gpsimd.tensor_tensor(hP[:, hsl], nt[:], zd[:], ALU.add)
                nc.vector.tensor_tensor(hd[:, hsl], zd[:], dt_[:], ALU.subtract)
                if blk > 1:
                    emit_tq(bt, blk - 2)
                if pending and blk % 2 == 1:
                    pending.pop(0)()
            emit_tq(bt, NB - 2)
            emit_tq(bt, NB - 1)
            if bt == 0:
                pending = attn_stages(0)
            else:
                while pending:
                    pending.pop(0)()
                for st in attn_stages(1):
                    st()

    nc.compile()
    return nc


_CACHE = {}


def _get_nc(has_gru_bias: bool):
    if has_gru_bias not in _CACHE:
        _CACHE[has_gru_bias] = _build(has_gru_bias)
    return _CACHE[has_gru_bias]


def _q8(x, scale):
    y = np.clip(np.asarray(x, np.float32) * scale, -240.0, 240.0)
    return np.ascontiguousarray(y).astype(F8)


def _prep(inputs):
    """Host-side sharding / layout prep. Returns (in_maps, has_gru_bias)."""
    inp = np.asarray(inputs["inp"], np.float32)
    hx = np.asarray(inputs["hx"], np.float32)
    has_gru_bias = bool(
        np.any(np.asarray(inputs["bih"])) or np.any(np.asarray(inputs["bhh"]))
        or np.any(np.asarray(inputs["fc_b"])) or np.any(np.asarray(inputs["gate_b"])))

    # ---- shared weight layouts (same for every core)
    Wk1 = np.asarray(inputs["Wk1"], np.float32)[1]            # (1024, 64)
    wk1 = Wk1.reshape(KI_IN, 128, 64).transpose(1, 0, 2).reshape(128, KI_IN * 64)
    wk1 = np.ascontiguousarray(wk1, np.float32)
    Wq1 = np.asarray(inputs["Wq1"], np.float32)               # (8, 256, 64)
    wq1t = np.ascontiguousarray(
        Wq1.transpose(2, 0, 1).reshape(64, NB * BS), np.float32)
    # W2[k] = Wv1[1] @ Wih[k]^T  (1024, 768) folded on host
    Wv1 = np.asarray(inputs["Wv1"], np.float32)[1]            # (1024, 1024)
    Wih = np.asarray(inputs["Wih"], np.float32)               # (8, 768, 1024)
    W2 = np.matmul(Wv1[None], Wih.transpose(0, 2, 1))         # (8, 1024, 768)
    w28 = _q8(
        W2.reshape(NB, KI_IN, 128, G3).transpose(2, 0, 1, 3)
        .reshape(128, NB * KI_IN * G3), S_W2)
    Whh = np.asarray(inputs["Whh"], np.float32)               # (8, 768, 256)
    whh = _q8(
        Whh.transpose(0, 2, 1).reshape(NB, 2, 128, G3)
        .transpose(2, 0, 1, 3).reshape(128, NB * 2 * G3), S_WHH)

    def proj_layout(w, scale=1.0):
        t = (np.asarray(w, np.float32) * scale).reshape(NB, 2, 128, 64)
        return np.ascontiguousarray(t.transpose(2, 0, 1, 3)
                                    .reshape(128, NB * 2, 64))

    wqkv = np.concatenate([proj_layout(inputs["Wq2"]),
                           proj_layout(inputs["Wk2"], 0.25),   # 1/sqrt(DK2)
                           proj_layout(inputs["Wv2"])], axis=2)
    wqkv = np.ascontiguousarray(wqkv.reshape(128, NB * 2 * 192)).astype(BF)
    fg = np.concatenate([np.asarray(inputs["fc_w"], np.float32),
                         np.asarray(inputs["gate_w"], np.float32)], axis=1)  # (64,512)
    fgw2 = np.zeros((128, 1024), np.float32)
    fgw2[0:64, 0:512] = fg
    fgw2[64:128, 512:1024] = fg
    fgw2 = fgw2.astype(BF)

    shared = dict(wk1=wk1, wq1t=wq1t, w28=w28, whh=whh, wqkv=wqkv, fgw2=fgw2)
    if has_gru_bias:
        bb = (np.asarray(inputs["bih"], np.float32)
              + np.asarray(inputs["bhh"], np.float32)) * (2.0 ** 14)  # (8,768)
        shared["bbB"] = np.ascontiguousarray(
            np.broadcast_to(bb.reshape(1, NB * G3), (128, NB * G3)), np.float32)
        shared["onesrow"] = np.ones((1, 128), BF)
        fgb = np.concatenate([np.asarray(inputs["fc_b"], np.float32),
                              np.asarray(inputs["gate_b"], np.float32)])
        shared["fgb2"] = np.concatenate([fgb, fgb]).reshape(1, 1024).astype(BF)

    in_maps = []
    for c in range(NCORES):
        r0 = c * BL
        inp_s = inp[r0:r0 + BL]                               # (256, 1024)
        hx_s = hx[r0:r0 + BL]                                 # (256, 2048)
        inpT = np.ascontiguousarray(
            inp_s.T.reshape(KI_IN, 128, BL).transpose(1, 0, 2)
            .reshape(128, KI_IN * BL), np.float32)
        # [p, bt*1024 + ki*128 + c] layouts for fp8/fp32 transposed tensors
        inpT8 = _q8(
            inp_s.T.reshape(KI_IN, 128, 2, 128).transpose(1, 2, 0, 3)
            .reshape(128, 2 * KI_IN * 128), S_INP)
        hxT4 = hx_s.T.reshape(KI_HID, 128, 2, 128).transpose(1, 2, 0, 3) \
            .reshape(128, 2 * KI_HID * 128)
        hxT8 = _q8(hxT4, S_INP)
        hxP = np.ascontiguousarray(
            hx_s.reshape(2, 128, NHID).transpose(1, 0, 2)
            .reshape(128, 2 * NHID), np.float32)
        m = dict(inpT=inpT, inpT8=inpT8, hxT8=hxT8, hxP=hxP, **shared)
        in_maps.append(m)
    return in_maps, has_gru_bias


_EXEC = {}


def _get_exec(nc, key):
    """Build (once) a cached jitted SPMD executor for `nc` (axon/PJRT path)."""
    if key in _EXEC:
        return _EXEC[key]
    import jax
    from jax.sharding import Mesh, PartitionSpec
    from jax.experimental.shard_map import shard_map
    from concourse import bass2jax
    from concourse.bass2jax import _bass_exec_p

    bass2jax.install_neuronx_cc_hook()

    partition_name = (nc.partition_id_tensor.name
                      if nc.partition_id_tensor else None)
    in_names, out_names, out_avals, zero_shapes = [], [], [], []
    for alloc in nc.m.functions[0].allocations:
        if not isinstance(alloc, mybir.MemoryLocationSet):
            continue
        name = alloc.memorylocations[0].name
        if alloc.kind == "ExternalInput":
            if name != partition_name:
                in_names.append(name)
        elif alloc.kind == "ExternalOutput":
            out_names.append(name)
            shape = tuple(alloc.tensor_shape)
            dtype = mybir.dt.np(alloc.dtype)
            out_avals.append(jax.core.ShapedArray(shape, dtype))
            zero_shapes.append((shape, dtype))
    n_params = len(in_names)
    all_names = list(in_names) + list(out_names)
    if partition_name is not None:
        all_names.append(partition_name)

    def _body(*args):
        operands = list(args)
        if partition_name is not None:
            operands.append(bass2jax.partition_id_tensor())
        outs = _bass_exec_p.bind(
            *operands,
            out_avals=tuple(out_avals),
            in_names=tuple(all_names),
            out_names=tuple(out_names),
            lowering_input_output_aliases=(),
            sim_require_finite=True,
            sim_require_nnan=True,
            nc=nc,
        )
        return tuple(outs)

    donate = tuple(range(n_params, n_params + len(out_names)))
    devices = jax.devices()[:NCORES]
    mesh = Mesh(np.asarray(devices), ("core",))
    in_specs = (PartitionSpec("core"),) * (n_params + len(out_names))
    out_specs = (PartitionSpec("core"),) * len(out_names)
    sharded = jax.jit(
        shard_map(_body, mesh=mesh, in_specs=in_specs, out_specs=out_specs,
                  check_rep=False),
        donate_argnums=donate, keep_unused=True)

    _EXEC[key] = (sharded, in_names, out_names, zero_shapes)
    return _EXEC[key]


def run_prepared(in_maps, has_gru_bias, iters=1):
    """Execute the compiled kernel on 8 cores; returns (per-core out arrays,
    list of per-iteration wall seconds)."""
    import time
    import jax
    from jax.sharding import NamedSharding, PartitionSpec
    nc = _get_nc(has_gru_bias)
    sharded, in_names, out_names, zero_shapes = _get_exec(nc, has_gru_bias)
    concat_in = [np.concatenate([np.asarray(m[n]) for m in in_maps], axis=0)
                 for n in in_names]
    times = []
    if iters > 1:
        from jax.sharding import Mesh
        mesh = Mesh(np.asarray(jax.devices()[:NCORES]), ("core",))
        sh = NamedSharding(mesh, PartitionSpec("core"))
        concat_in = [jax.device_put(a, sh) for a in concat_in]
        zero_sets = []
        for _ in range(iters):
            zero_sets.append([
                jax.device_put(np.zeros((NCORES * s[0], *s[1:]), d), sh)
                for s, d in zero_shapes])
        jax.block_until_ready(concat_in)
        jax.block_until_ready(zero_sets)
        out_arrs = sharded(*concat_in, *zero_sets[0])
        jax.block_until_ready(out_arrs)
        t0 = time.perf_counter()
        for i in range(1, iters):
            out_arrs = sharded(*concat_in, *zero_sets[i])
        jax.block_until_ready(out_arrs)
        dt = (time.perf_counter() - t0) / (iters - 1)
        times = [dt] * iters
        out_arrs = [np.asarray(a) for a in out_arrs]
    else:
        zeros = [np.zeros((NCORES * s[0], *s[1:]), d) for s, d in zero_shapes]
        t0 = time.perf_counter()
        out_arrs = sharded(*concat_in, *zeros)
        jax.block_until_ready(out_arrs)
        out_arrs = [np.asarray(a) for a in out_arrs]
        times.append(time.perf_counter() - t0)
    i = out_names.index("out")
    j = out_names.index("maskout")
    full = out_arrs[i].reshape(NCORES, 2, 128, NHID)
    mfull = out_arrs[j].reshape(NCORES, 128, 16)
    return (full, mfull), times


def kernel(**inputs):
    in_maps, has_gru_bias = _prep(inputs)
    (full, mfull), _ = run_prepared(in_maps, has_gru_bias, iters=1)
    res = np.empty((B, NHID), np.float32)
    mask_blk = np.empty((B, NB), np.float32)
    for c in range(NCORES):
        res[c * BL:(c + 1) * BL] = full[c].reshape(BL, NHID)
        for bt in range(2):
            mask_blk[c * BL + bt * 128: c * BL + (bt + 1) * 128] = \
                mfull[c][:, bt * 8:(bt + 1) * 8]
    mask = np.repeat(mask_blk, BS, axis=1)
    return res, mask


# revision 44
# speedup vs baseline: 1.1467x; 1.0660x over previous
"""Trainium2 Bass kernel for nn_BlocksCore (RIMs-style BlocksCore forward).

Sharding: data-parallel over batch B=2048 across 8 NeuronCores (256 rows each,
zero cross-core communication; all model ops are batch-independent).

Key optimizations over the bf16 baseline:
  - W2 = Wv1[1] @ Wih^T is folded on the host, so the GRU input projection
    gi = s * (inp @ W2) consumes inp directly: the whole v1 matmul stage
    disappears and one fp8 quantization step is avoided.
  - gi and gh run as fp8e4 DoubleRow matmuls (2 k-subtiles per pass). Both
    arrive in PSUM scaled by 2^14 (inp*16 x W2*4096 / 16, hx*16 x Whh*1024),
    so s*gi+gh combines with one STT and the 2^-14 descale rides the
    sigmoid/tanh activation scale for free.
  - Routing stays fp32 (top-k margins are ~1e-6) but restructured: k1 and
    q = hx@Wq1 are computed row-major on the PE (half the fp32 columns of
    the old k1T/wp form) and the 64-dim dot runs on DVE.
  - bt-major pipeline: all 8 blocks of batch-half 0 run first, then its
    comm-attention chain (Vector-heavy) is emitted so it overlaps with
    batch-half 1's GRU matmuls on the PE.
  - fc and gate projections share one block-diagonal stationary matrix
    (fgw2), so each q-pair needs a single K=128 matmul instead of two
    half-wasted K=64 matmuls.
  - Elementwise work is spread V/S/G: STT gate-combines and small TTs on
    Vector, sigmoid/tanh/copies on Scalar, h' blend products on GpSimd.
"""
import sys

sys.path.insert(0, "/opt/trn_rl_repo")

import numpy as np
import ml_dtypes

import concourse.bass as bass
import concourse.tile as tile
from concourse.masks import make_identity
from concourse import bacc, mybir

f32 = mybir.dt.float32
bf16 = mybir.dt.bfloat16
fp8 = mybir.dt.float8e4
AF = mybir.ActivationFunctionType
ALU = mybir.AluOpType
AX = mybir.AxisListType
DR = mybir.MatmulPerfMode.DoubleRow
DRS = mybir.MatmulPerfMode.DoubleRowSwInterleave

B, NINP, NHID = 2048, 1024, 2048
NB, BS, G3 = 8, 256, 768          # blocks, block_size_out, 3*BS
NH2, DK2, DV2, HD = 4, 16, 16, 64  # comm attn heads, dims, NH2*DV2
NCORES = 8
BL = B // NCORES                   # 256 rows per core
KI_IN = NINP // 128                # 8
KI_HID = NHID // 128               # 16

BF = ml_dtypes.bfloat16
F8 = ml_dtypes.float8_e4m3

# power-of-two quantization scales (values clipped to +-240 on host)
S_INP = 16.0      # inp, hx fp8 copies
S_W2 = 1024.0     # W2 = Wv1 @ Wih^T
S_WHH = 1024.0
# pu = (inp*16) @ (W2*1024) = gi * 2^14 ; pvh = (hx*16) @ (Whh*1024) = gh * 2^14
PSC = 2.0 ** -14  # descale applied inside sigmoid/tanh activation scale

# DoubleRowSwInterleave: stationary pre-interleaved on host (pairs adjacent,
# columns reversed) so the weight load reads contiguously (keeps FWL).
SWI = True


def _vap(sl, dims):
    """Custom free-dim view: keep partition dim of slice `sl`, replace free dims."""
    return bass.AP(sl.tensor, sl.offset, [sl.ap[0]] + [list(d) for d in dims])


def _build(has_gru_bias: bool):
    nc = bacc.Bacc("TRN2", target_bir_lowering=False, debug=False,
                   num_devices=NCORES)

    def din(name, shape, dt):
        return nc.dram_tensor(name, list(shape), dt, kind="ExternalInput").ap()

    inpT_d = din("inpT", (128, KI_IN * BL), f32)          # [p, ki*256+c]
    inpT8_d = din("inpT8", (128, 2 * KI_IN * 128), fp8)   # [p, bt*1024+ki*128+c]
    hxP_d = din("hxP", (128, 2 * NHID), f32)              # [p, bt*2048+f]
    hxT8_d = din("hxT8", (128, 2 * KI_HID * 128), fp8)    # [p, bt*2048+t*128+c]
    wk1_d = din("wk1", (128, KI_IN * 64), f32)            # [p, ki*64+d]
    wq1t_d = din("wq1t", (64, NB * BS), f32)              # [d, blk*256+f]
    w28_d = din("w28", (128, NB * KI_IN * G3), fp8)       # [p, blk*6144+ki*768+g]
    whh_d = din("whh", (128, NB * 2 * G3), fp8)           # [p, blk*1536+ki*768+g]
    wqkv_d = din("wqkv", (128, NB * 2 * 192), bf16)       # [p, t*192 + {q2|k2/4|v2}]
    fgw2_d = din("fgw2", (128, 1024), bf16)               # block-diag [fc|gate; fc|gate]
    if has_gru_bias:
        bbB_d = din("bbB", (128, NB * G3), f32)           # (bih+bhh)*2^14 bcast
        ones_d = din("onesrow", (1, 128), bf16)
        fgb2_d = din("fgb2", (1, 1024), bf16)             # fc_b|gate_b twice
    out_d = nc.dram_tensor("out", [2, 128, NHID], f32, kind="ExternalOutput").ap()
    mask_d = nc.dram_tensor("maskout", [128, 16], f32, kind="ExternalOutput").ap()

    from contextlib import ExitStack
    with tile.TileContext(nc) as tc, ExitStack() as ctx:
        P = ctx.enter_context(tc.tile_pool(name="persist", bufs=1))
        p32 = ctx.enter_context(tc.tile_pool(name="p32", bufs=2))
        gwork = ctx.enter_context(tc.tile_pool(name="gwork", bufs=2))
        prodp = ctx.enter_context(tc.tile_pool(name="prodp", bufs=1))
        tailp = ctx.enter_context(tc.tile_pool(name="tailp", bufs=2))
        # PSUM budget, bank-granular (8x2KB): pu 2x2 + pvh 1x2 + sm 1x1 + fg 1x1
        ps_u = ctx.enter_context(tc.tile_pool(name="ps_u", bufs=2, space="PSUM"))
        ps_v = ctx.enter_context(tc.tile_pool(name="ps_v", bufs=1, space="PSUM"))
        ps_sm = ctx.enter_context(tc.tile_pool(name="ps_sm", bufs=1, space="PSUM"))
        ps_fg = ctx.enter_context(tc.tile_pool(name="ps_fg", bufs=1, space="PSUM"))

        # ---- persistent sbuf tensors
        inpT8 = P.tile([128, 2 * KI_IN * 128], fp8, tag="inpT8")
        hxP_f = P.tile([128, 2 * NHID], f32, tag="hxP_f")
        hxT8 = P.tile([128, 2 * KI_HID * 128], fp8, tag="hxT8")
        wk1 = P.tile([128, KI_IN * 64], f32, tag="wk1")
        wq1t = P.tile([64, NB * BS], f32, tag="wq1t")
        w28 = P.tile([128, NB * KI_IN * G3], fp8, tag="w28")
        whh = P.tile([128, NB * 2 * G3], fp8, tag="whh")
        wqkv = P.tile([128, NB * 2 * 192], bf16, tag="wqkv")
        fgw2 = P.tile([128, 1024], bf16, tag="fgw2")
        k1T = P.tile([64, BL], f32, tag="k1T")
        aP = P.tile([128, 16], f32, tag="aP")
        sS = P.tile([128, 16], f32, tag="sS")
        mS = P.tile([128, 16], f32, tag="mS")
        cnt = P.tile([128, 16], f32, tag="cnt")
        cmp_t = P.tile([128, 128], f32, tag="cmp")
        hP = P.tile([128, 2 * NHID], bf16, tag="hP")
        hd = P.tile([128, 2 * NHID], bf16, tag="hd")
        hT = P.tile([128, KI_HID * BL], bf16, tag="hT")
        qk2P = P.tile([128, 2 * 1024], bf16, tag="qk2P")   # [bt*1024 + {0:q2,512:k2} + blk*64]
        v2P = P.tile([128, 2 * NB * 64], bf16, tag="v2P")  # [bt*512 + h*128 + d*8 + k]
        Lp = P.tile([128, 2 * 256], bf16, tag="Lp")
        attE = P.tile([128, 2 * 256], bf16, tag="attE")
        attS = P.tile([128, 2 * 32], bf16, tag="attS")
        attR = P.tile([128, 2 * 32], bf16, tag="attR")
        out2P = P.tile([128, 2 * NB * 64], bf16, tag="out2P")
        out2T = P.tile([128, 4 * BL], bf16, tag="out2T")  # [(q%2)*64+hd, (q//2)*256+bt*128+b]
        if has_gru_bias:
            bbB = P.tile([128, NB * G3], f32, tag="bbB")
            onesR = P.tile([1, 128], bf16, tag="onesR")
            fgb2 = P.tile([1, 1024], bf16, tag="fgb2")

        inpT_f = p32.tile([128, KI_IN * BL], f32, tag="big32")

        # ---- DMA emission. sync queue: routing-critical inp stream + fp8 GRU
        # weights; scalar queue: hx stream + attn weights.
        nc.sync.dma_start(wk1[:], wk1_d[:])
        nc.sync.dma_start(inpT_f[:], inpT_d[:])
        nc.sync.dma_start(inpT8[:], inpT8_d[:])
        nc.sync.dma_start(whh[:], whh_d[:])
        for blk in range(NB):
            nc.sync.dma_start(w28[:, blk * 6144:(blk + 1) * 6144],
                              w28_d[:, blk * 6144:(blk + 1) * 6144])
        nc.scalar.dma_start(wq1t[:], wq1t_d[:])
        nc.scalar.dma_start(hxP_f[:, 0:NHID], hxP_d[:, 0:NHID])
        nc.scalar.dma_start(hxT8[:], hxT8_d[:])
        nc.scalar.dma_start(hxP_f[:, NHID:], hxP_d[:, NHID:])
        nc.scalar.dma_start(wqkv[:], wqkv_d[:])
        nc.scalar.dma_start(fgw2[:], fgw2_d[:])
        if has_gru_bias:
            nc.scalar.dma_start(bbB[:], bbB_d[:])
            nc.scalar.dma_start(onesR[:], ones_d[:])
            nc.scalar.dma_start(fgb2[:], fgb2_d[:])

        # ---- routing (all fp32, wide-N matmuls): k1T = (inp@Wk1)^T, then
        # a[row,blk] = sum_f (k1T^T @ Wq1^T)[row,f] * hx[row,f] via STT accum.
        kp = ps_sm.tile([64, BL], f32, tag="sm")
        for ki in range(KI_IN):
            nc.tensor.matmul(kp[:], wk1[:, ki * 64:(ki + 1) * 64],
                             inpT_f[:, ki * BL:(ki + 1) * BL],
                             start=(ki == 0), stop=(ki == KI_IN - 1))
        nc.vector.tensor_copy(k1T[:], kp[:])
        for bt in range(2):
            for b2 in range(4):  # block pairs, one N=512 fp32 matmul each
                wp = ps_fg.tile([128, 512], f32, tag="fg")
                nc.tensor.matmul(wp[:], k1T[:, bt * 128:(bt + 1) * 128],
                                 wq1t[:, b2 * 512:(b2 + 1) * 512],
                                 start=True, stop=True)
                for j in range(2):
                    blk = 2 * b2 + j
                    scr = gwork.tile([128, BS], f32, tag="scr")
                    col = bt * 8 + blk
                    nc.vector.scalar_tensor_tensor(
                        scr[:], wp[:, j * BS:(j + 1) * BS], 0.125,
                        hxP_f[:, bt * NHID + blk * BS: bt * NHID + (blk + 1) * BS],
                        ALU.mult, ALU.mult, accum_out=aP[:, col:col + 1])
        nc.scalar.activation(sS[:], aP[:], AF.Sigmoid)
        # mask: cnt[bt,k] = #{j : a[bt,j] > a[bt,k]};  keep iff cnt < 4
        i0 = _vap(aP[:], [[8, 2], [1, 8], [0, 8]])
        i1 = _vap(aP[:], [[8, 2], [0, 8], [1, 8]])
        ov = _vap(cmp_t[:], [[64, 2], [1, 8], [8, 8]])
        nc.vector.tensor_tensor(ov, i0, i1, ALU.is_gt)
        rin = _vap(cmp_t[:], [[64, 2], [8, 8], [1, 8]])
        nc.vector.reduce_sum(cnt[:], rin, axis=AX.X)
        nc.vector.tensor_scalar(mS[:], cnt[:], 3.5, None, ALU.is_lt)
        nc.scalar.dma_start(mask_d[:], mS[:])

        # ---- per-block GRU with fp8 DoubleRow matmuls (pu = gi*2^14,
        # pvh = gh*2^14); transposes + qkv projection pipelined one block late.
        def emit_tq(bt, blk):
            for ki in range(2):
                ft = blk * 2 + ki
                nc.sync.dma_start_transpose(
                    hT[:, ft * BL + bt * 128: ft * BL + (bt + 1) * 128],
                    hP[:, bt * NHID + ft * 128: bt * NHID + (ft + 1) * 128])
            pqkv = ps_sm.tile([128, 192], f32, tag="sm")
            for ki in range(2):
                t_idx = blk * 2 + ki
                lhs = hT[:, t_idx * BL + bt * 128: t_idx * BL + (bt + 1) * 128]
                nc.tensor.matmul(pqkv[:], lhs,
                                 wqkv[:, t_idx * 192:(t_idx + 1) * 192],
                                 start=(ki == 0), stop=(ki == 1))
            qb = qk2P[:, bt * 1024 + blk * 64: bt * 1024 + blk * 64 + 1]
            nc.scalar.activation(_vap(qb, [[512, 2], [1, 64]]),
                                 pqkv[:, 0:128], AF.Copy)
            vsl = v2P[:, bt * 512 + blk: bt * 512 + blk + 1]
            nc.scalar.activation(_vap(vsl, [[128, 4], [8, 16]]),
                                 pqkv[:, 128:192], AF.Copy)

        def attn_stages(bt):
            # comm attention for one 128-row half, split into 4 dispensable
            # stages so they interleave with the other half's GRU emission.
            outS = p32.tile([128, NHID], f32, tag="big32")
            l0 = bt * 256
            s0 = bt * 32

            def s_pass1():
                with nc.allow_low_precision("bf16 comm-attn accumulation"):
                    pr = prodp.tile([128, 4096], bf16, tag="pr")
                    q0 = bt * 1024
                    i0 = _vap(qk2P[:, q0:q0 + 1],
                              [[64, 8], [0, 8], [16, 4], [1, 16]])
                    i1 = _vap(qk2P[:, bt * 1024 + 512: bt * 1024 + 513],
                              [[0, 8], [64, 8], [16, 4], [1, 16]])
                    ovp = _vap(pr[:], [[512, 8], [64, 8], [16, 4], [1, 16]])
                    nc.vector.tensor_tensor(ovp, i0, i1, ALU.mult)
                    rin = _vap(pr[:], [[512, 8], [16, 4], [64, 8], [1, 16]])
                    lo = _vap(Lp[:, l0:l0 + 1], [[32, 8], [8, 4], [1, 8]])
                    nc.vector.reduce_sum(lo, rin, axis=AX.X)
                    esl = slice(l0, l0 + 256)
                    nc.scalar.activation(attE[:, esl], Lp[:, esl], AF.Exp)
                    sin = _vap(attE[:, l0:l0 + 1], [[32, 8], [8, 4], [1, 8]])
                    so = _vap(attS[:, s0:s0 + 1], [[4, 8], [1, 4]])
                    nc.vector.reduce_sum(so, sin, axis=AX.X)
                    nc.vector.reciprocal(attR[:, s0:s0 + 32], attS[:, s0:s0 + 32])

            def s_pass2():
                # unnormalized values (attE weights); softmax 1/sum folded in
                # after the k-reduce, shortening the serial chain.
                with nc.allow_low_precision("bf16 comm-attn accumulation"):
                    pv_ = prodp.tile([128, 4096], bf16, tag="pr")
                    av = _vap(attE[:, l0:l0 + 1],
                              [[32, 8], [8, 4], [0, 16], [1, 8]])
                    vv = _vap(v2P[:, bt * 512: bt * 512 + 1],
                              [[0, 8], [128, 4], [8, 16], [1, 8]])
                    pvv = _vap(pv_[:], [[512, 8], [128, 4], [8, 16], [1, 8]])
                    nc.vector.tensor_tensor(pvv, av, vv, ALU.mult)
                    o0 = bt * 512
                    o2u = gwork.tile([128, 512], bf16, tag="o2u")
                    o2uv = _vap(o2u[:, 0:1], [[64, 8], [16, 4], [1, 16]])
                    nc.vector.reduce_sum(
                        o2uv, _vap(pv_[:], [[512, 8], [128, 4], [8, 16], [1, 8]]),
                        axis=AX.X)
                    rv = _vap(attR[:, s0:s0 + 1], [[1, 4], [0, 16], [4, 8]])
                    o2o = _vap(out2P[:, bt * 512: bt * 512 + 1],
                               [[16, 4], [1, 16], [64, 8]])
                    o2in = _vap(o2u[:, 0:1], [[16, 4], [1, 16], [64, 8]])
                    nc.vector.tensor_tensor(o2o, o2in, rv, ALU.mult)
                    for qp_i in range(4):
                        eng = nc.sync if qp_i % 2 == 0 else nc.scalar
                        eng.dma_start_transpose(
                            out2T[:, qp_i * 256 + bt * 128: qp_i * 256 + (bt + 1) * 128],
                            out2P[:, bt * 512 + qp_i * 128: bt * 512 + (qp_i + 1) * 128])

            def s_fg(g2a, g2b, dma):
                # fc|gate: block-diag fgw2 split into two 1-bank N=512 matmuls
                # (cols 0:512 hit only rows 0:64 = q-even, 512:1024 = q-odd)
                for g2 in range(g2a, g2b):
                    c0 = g2 * 256 + bt * 128
                    for j in range(2):  # q = 2*g2 + j
                        q = 2 * g2 + j
                        pfg = ps_fg.tile([128, 512], f32, tag="fg")
                        nc.tensor.matmul(pfg[:], out2T[:, c0:c0 + 128],
                                         fgw2[:, j * 512:(j + 1) * 512],
                                         start=True, stop=not has_gru_bias)
                        if has_gru_bias:
                            nc.tensor.matmul(pfg[:], onesR[:],
                                             fgb2[:, j * 512:(j + 1) * 512],
                                             start=False, stop=True)
                        gt = tailp.tile([128, BS], bf16, tag="gt")
                        ft_ = tailp.tile([128, BS], bf16, tag="ft")
                        nc.scalar.activation(gt[:], pfg[:, 256:512], AF.Sigmoid)
                        nc.scalar.activation(ft_[:], pfg[:, 0:256], AF.Tanh)
                        hatt = tailp.tile([128, BS], bf16, tag="hatt")
                        nc.gpsimd.tensor_tensor(hatt[:], gt[:], ft_[:], ALU.mult)
                        hx_sl = slice(bt * NHID + q * BS, bt * NHID + (q + 1) * BS)
                        d2 = tailp.tile([128, BS], bf16, tag="d2")
                        nc.gpsimd.tensor_tensor(d2[:], hd[:, hx_sl], hatt[:], ALU.add)
                        qcol = bt * 8 + q
                        nc.vector.scalar_tensor_tensor(
                            outS[:, q * BS:(q + 1) * BS], d2[:],
                            mS[:, qcol:qcol + 1], hxP_f[:, hx_sl],
                            ALU.mult, ALU.add)
                if g2b == 2:
                    nc.scalar.dma_start(out_d[bt][:, 0:1024], outS[:, 0:1024])
                if dma:
                    nc.scalar.dma_start(out_d[bt][:, 1024:2048], outS[:, 1024:2048])

            return [s_pass1, s_pass2,
                    lambda: s_fg(0, 2, False), lambda: s_fg(2, 4, True)]

        pending = []   # attn stages of the previous bt, dispensed into this loop
        for bt in range(2):
            for blk in range(NB):
                col = bt * 8 + blk
                s_col = sS[:, col:col + 1]
                # gi: 4 DoubleRow steps over ki-pairs, two PSUM regions
                pu = ps_u.tile([128, G3], f32, tag="pu")
                for kk in range(4):
                    ki = 2 * kk
                    off = bt * 1024 + ki * 128
                    if SWI:
                        lhsT = _vap(inpT8[:, off:off + 1], [[2, 128], [1, 2]])
                    else:
                        lhsT = _vap(inpT8[:, off:off + 1], [[128, 2], [1, 128]])
                    w0 = blk * 6144 + ki * G3
                    nc.tensor.matmul(pu[:, 0:512], lhsT,
                                     _vap(w28[:, w0:w0 + 1], [[G3, 2], [1, 512]]),
                                     start=(kk == 0), stop=(kk == 3),
                                     perf_mode=DRS if SWI else DR)
                    nc.tensor.matmul(pu[:, 512:G3], lhsT,
                                     _vap(w28[:, w0 + 512:w0 + 513], [[G3, 2], [1, 256]]),
                                     start=(kk == 0), stop=(kk == 3),
                                     perf_mode=DRS if SWI else DR)
                # gh: one DoubleRow step (contraction 256 = whole block)
                pvh = ps_v.tile([128, G3], f32, tag="pvh")
                hx0 = bt * 2048 + blk * 2 * 128
                if SWI:
                    lhsT = _vap(hxT8[:, hx0:hx0 + 1], [[2, 128], [1, 2]])
                else:
                    lhsT = _vap(hxT8[:, hx0:hx0 + 1], [[128, 2], [1, 128]])
                wh0 = blk * 2 * G3
                nc.tensor.matmul(pvh[:, 0:512], lhsT,
                                 _vap(whh[:, wh0:wh0 + 1], [[G3, 2], [1, 512]]),
                                 start=True, stop=True, perf_mode=DRS if SWI else DR)
                nc.tensor.matmul(pvh[:, 512:G3], lhsT,
                                 _vap(whh[:, wh0 + 512:wh0 + 513], [[G3, 2], [1, 256]]),
                                 start=True, stop=True, perf_mode=DRS if SWI else DR)
                # gates: rz = sigmoid((s*gi_rz + gh_rz) * 2^-14).  STT cannot
                # read two PSUM operands, so gh_rz goes through SBUF first.
                pvc = gwork.tile([128, G3], bf16, tag="pvc")
                nc.scalar.activation(pvc[:], pvh[:], AF.Copy)
                rzp = gwork.tile([128, 512], f32, tag="rzp")
                nc.vector.scalar_tensor_tensor(
                    rzp[:], pu[:, 0:512], s_col, pvc[:, 0:512], ALU.mult, ALU.add)
                if has_gru_bias:
                    nc.vector.tensor_tensor(rzp[:], rzp[:],
                                            bbB[:, blk * G3: blk * G3 + 512], ALU.add)
                rzs = gwork.tile([128, 512], bf16, tag="rzs")
                nc.scalar.activation(rzs[:], rzp[:], AF.Sigmoid, scale=PSC)
                rhn = gwork.tile([128, BS], bf16, tag="rhn")
                nc.vector.tensor_tensor(rhn[:], rzs[:, 0:BS], pvc[:, 512:G3], ALU.mult)
                npre = gwork.tile([128, BS], f32, tag="npre")
                nc.vector.scalar_tensor_tensor(
                    npre[:], pu[:, 512:G3], s_col, rhn[:], ALU.mult, ALU.add)
                if has_gru_bias:
                    nc.vector.tensor_tensor(
                        npre[:], npre[:],
                        bbB[:, blk * G3 + 512: (blk + 1) * G3], ALU.add)
                nt = gwork.tile([128, BS], bf16, tag="nt")
                nc.scalar.activation(nt[:], npre[:], AF.Tanh, scale=PSC)
                # h' = n + z*(h-n);  hd = h' - h = (z-1)*(n-h)... = zd - dt
                hsl = slice(bt * NHID + blk * BS, bt * NHID + (blk + 1) * BS)
                dt_ = gwork.tile([128, BS], bf16, tag="dt")
                nc.gpsimd.tensor_tensor(dt_[:], hxP_f[:, hsl], nt[:], ALU.subtract)
                zd = gwork.tile([128, BS], bf16, tag="zd")
                nc.gpsimd.tensor_tensor(zd[:], rzs[:, BS:512], dt_[:], ALU.mult)
                nc.gpsimd.tensor_tensor(hP[:, hsl], nt[:], zd[:], ALU.add)
                nc.vector.tensor_tensor(hd[:, hsl], zd[:], dt_[:], ALU.subtract)
                if blk > 1:
                    emit_tq(bt, blk - 2)
                if pending and blk % 2 == 1:
                    pending.pop(0)()
            emit_tq(bt, NB - 2)
            emit_tq(bt, NB - 1)
            if bt == 0:
                pending = attn_stages(0)
            else:
                while pending:
                    pending.pop(0)()
                for st in attn_stages(1):
                    st()

    nc.compile()
    return nc


_CACHE = {}


def _get_nc(has_gru_bias: bool):
    if has_gru_bias not in _CACHE:
        _CACHE[has_gru_bias] = _build(has_gru_bias)
    return _CACHE[has_gru_bias]


def _q8(x, scale):
    y = np.clip(np.asarray(x, np.float32) * scale, -240.0, 240.0)
    return np.ascontiguousarray(y).astype(F8)


def _prep(inputs):
    """Host-side sharding / layout prep. Returns (in_maps, has_gru_bias)."""
    inp = np.asarray(inputs["inp"], np.float32)
    hx = np.asarray(inputs["hx"], np.float32)
    has_gru_bias = bool(
        np.any(np.asarray(inputs["bih"])) or np.any(np.asarray(inputs["bhh"]))
        or np.any(np.asarray(inputs["fc_b"])) or np.any(np.asarray(inputs["gate_b"])))

    # ---- shared weight layouts (same for every core)
    Wk1 = np.asarray(inputs["Wk1"], np.float32)[1]            # (1024, 64)
    wk1 = Wk1.reshape(KI_IN, 128, 64).transpose(1, 0, 2).reshape(128, KI_IN * 64)
    wk1 = np.ascontiguousarray(wk1, np.float32)
    Wq1 = np.asarray(inputs["Wq1"], np.float32)               # (8, 256, 64)
    wq1t = np.ascontiguousarray(
        Wq1.transpose(2, 0, 1).reshape(64, NB * BS), np.float32)
    # W2[k] = Wv1[1] @ Wih[k]^T  (1024, 768) folded on host
    Wv1 = np.asarray(inputs["Wv1"], np.float32)[1]            # (1024, 1024)
    Wih = np.asarray(inputs["Wih"], np.float32)               # (8, 768, 1024)
    W2 = np.matmul(Wv1[None], Wih.transpose(0, 2, 1))         # (8, 1024, 768)
    w28 = _q8(
        W2.reshape(NB, KI_IN, 128, G3).transpose(2, 0, 1, 3)
        .reshape(128, NB * KI_IN * G3), S_W2)
    Whh = np.asarray(inputs["Whh"], np.float32)               # (8, 768, 256)
    whh = _q8(
        Whh.transpose(0, 2, 1).reshape(NB, 2, 128, G3)
        .transpose(2, 0, 1, 3).reshape(128, NB * 2 * G3), S_WHH)

    def proj_layout(w, scale=1.0):
        t = (np.asarray(w, np.float32) * scale).reshape(NB, 2, 128, 64)
        return np.ascontiguousarray(t.transpose(2, 0, 1, 3)
                                    .reshape(128, NB * 2, 64))

    wqkv = np.concatenate([proj_layout(inputs["Wq2"]),
                           proj_layout(inputs["Wk2"], 0.25),   # 1/sqrt(DK2)
                           proj_layout(inputs["Wv2"])], axis=2)
    wqkv = np.ascontiguousarray(wqkv.reshape(128, NB * 2 * 192)).astype(BF)
    fg = np.concatenate([np.asarray(inputs["fc_w"], np.float32),
                         np.asarray(inputs["gate_w"], np.float32)], axis=1)  # (64,512)
    fgw2 = np.zeros((128, 1024), np.float32)
    fgw2[0:64, 0:512] = fg
    fgw2[64:128, 512:1024] = fg
    fgw2 = fgw2.astype(BF)

    shared = dict(wk1=wk1, wq1t=wq1t, w28=w28, whh=whh, wqkv=wqkv, fgw2=fgw2)
    if has_gru_bias:
        bb = (np.asarray(inputs["bih"], np.float32)
              + np.asarray(inputs["bhh"], np.float32)) * (2.0 ** 14)  # (8,768)
        shared["bbB"] = np.ascontiguousarray(
            np.broadcast_to(bb.reshape(1, NB * G3), (128, NB * G3)), np.float32)
        shared["onesrow"] = np.ones((1, 128), BF)
        fgb = np.concatenate([np.asarray(inputs["fc_b"], np.float32),
                              np.asarray(inputs["gate_b"], np.float32)])
        shared["fgb2"] = np.concatenate([fgb, fgb]).reshape(1, 1024).astype(BF)

    in_maps = []
    for c in range(NCORES):
        r0 = c * BL
        inp_s = inp[r0:r0 + BL]                               # (256, 1024)
        hx_s = hx[r0:r0 + BL]                                 # (256, 2048)
        inpT = np.ascontiguousarray(
            inp_s.T.reshape(KI_IN, 128, BL).transpose(1, 0, 2)
            .reshape(128, KI_IN * BL), np.float32)
        if SWI:
            # [p, bt, kk, j, t]: pair j holds (ki=2kk, 2kk+1) values for
            # batch column (127 - j)  (SwInterleave ISA layout)
            X = inp_s.T.reshape(4, 2, 128, 2, 128)       # [kk, t, p, bt, c]
            inpT8 = _q8(X[..., ::-1].transpose(2, 3, 0, 4, 1)
                        .reshape(128, 2 * KI_IN * 128), S_INP)
            H = hx_s.T.reshape(8, 2, 128, 2, 128)        # [blk, t, p, bt, c]
            hxT8 = _q8(H[..., ::-1].transpose(2, 3, 0, 4, 1)
                       .reshape(128, 2 * KI_HID * 128), S_INP)
        else:
            # [p, bt*1024 + ki*128 + c] layouts for fp8 transposed tensors
            inpT8 = _q8(
                inp_s.T.reshape(KI_IN, 128, 2, 128).transpose(1, 2, 0, 3)
                .reshape(128, 2 * KI_IN * 128), S_INP)
            hxT8 = _q8(
                hx_s.T.reshape(KI_HID, 128, 2, 128).transpose(1, 2, 0, 3)
                .reshape(128, 2 * KI_HID * 128), S_INP)
        hxP = np.ascontiguousarray(
            hx_s.reshape(2, 128, NHID).transpose(1, 0, 2)
            .reshape(128, 2 * NHID), np.float32)
        m = dict(inpT=inpT, inpT8=inpT8, hxT8=hxT8, hxP=hxP, **shared)
        in_maps.append(m)
    return in_maps, has_gru_bias


_EXEC = {}


def _get_exec(nc, key):
    """Build (once) a cached jitted SPMD executor for `nc` (axon/PJRT path)."""
    if key in _EXEC:
        return _EXEC[key]
    import jax
    from jax.sharding import Mesh, PartitionSpec
    from jax.experimental.shard_map import shard_map
    from concourse import bass2jax
    from concourse.bass2jax import _bass_exec_p

    bass2jax.install_neuronx_cc_hook()

    partition_name = (nc.partition_id_tensor.name
                      if nc.partition_id_tensor else None)
    in_names, out_names, out_avals, zero_shapes = [], [], [], []
    for alloc in nc.m.functions[0].allocations:
        if not isinstance(alloc, mybir.MemoryLocationSet):
            continue
        name = alloc.memorylocations[0].name
        if alloc.kind == "ExternalInput":
            if name != partition_name:
                in_names.append(name)
        elif alloc.kind == "ExternalOutput":
            out_names.append(name)
            shape = tuple(alloc.tensor_shape)
            dtype = mybir.dt.np(alloc.dtype)
            out_avals.append(jax.core.ShapedArray(shape, dtype))
            zero_shapes.append((shape, dtype))
    n_params = len(in_names)
    all_names = list(in_names) + list(out_names)
    if partition_name is not None:
        all_names.append(partition_name)

    def _body(*args):
        operands = list(args)
        if partition_name is not None:
            operands.append(bass2jax.partition_id_tensor())
        outs = _bass_exec_p.bind(
            *operands,
            out_avals=tuple(out_avals),
            in_names=tuple(all_names),
            out_names=tuple(out_names),
            lowering_input_output_aliases=(),
            sim_require_finite=True,
            sim_require_nnan=True,
            nc=nc,
        )
        return tuple(outs)

    donate = tuple(range(n_params, n_params + len(out_names)))
    devices = jax.devices()[:NCORES]
    mesh = Mesh(np.asarray(devices), ("core",))
    in_specs = (PartitionSpec("core"),) * (n_params + len(out_names))
    out_specs = (PartitionSpec("core"),) * len(out_names)
    sharded = jax.jit(
        shard_map(_body, mesh=mesh, in_specs=in_specs, out_specs=out_specs,
                  check_rep=False),
        donate_argnums=donate, keep_unused=True)

    _EXEC[key] = (sharded, in_names, out_names, zero_shapes)
    return _EXEC[key]


def run_prepared(in_maps, has_gru_bias, iters=1):
    """Execute the compiled kernel on 8 cores; returns (per-core out arrays,
    list of per-iteration wall seconds)."""
    import time
    import jax
    from jax.sharding import NamedSharding, PartitionSpec
    nc = _get_nc(has_gru_bias)
    sharded, in_names, out_names, zero_shapes = _get_exec(nc, has_gru_bias)
    concat_in = [np.concatenate([np.asarray(m[n]) for m in in_maps], axis=0)
                 for n in in_names]
    times = []
    if iters > 1:
        from jax.sharding import Mesh
        mesh = Mesh(np.asarray(jax.devices()[:NCORES]), ("core",))
        sh = NamedSharding(mesh, PartitionSpec("core"))
        concat_in = [jax.device_put(a, sh) for a in concat_in]
        zero_sets = []
        for _ in range(iters):
            zero_sets.append([
                jax.device_put(np.zeros((NCORES * s[0], *s[1:]), d), sh)
                for s, d in zero_shapes])
        jax.block_until_ready(concat_in)
        jax.block_until_ready(zero_sets)
        out_arrs = sharded(*concat_in, *zero_sets[0])
        jax.block_until_ready(out_arrs)
        t0 = time.perf_counter()
        for i in range(1, iters):
            out_arrs = sharded(*concat_in, *zero_sets[i])
        jax.block_until_ready(out_arrs)
        dt = (time.perf_counter() - t0) / (iters - 1)
        times = [dt] * iters
        out_arrs = [np.asarray(a) for a in out_arrs]
    else:
        zeros = [np.zeros((NCORES * s[0], *s[1:]), d) for s, d in zero_shapes]
        t0 = time.perf_counter()
        out_arrs = sharded(*concat_in, *zeros)
        jax.block_until_ready(out_arrs)
        out_arrs = [np.asarray(a) for a in out_arrs]
        times.append(time.perf_counter() - t0)
    i = out_names.index("out")
    j = out_names.index("maskout")
    full = out_arrs[i].reshape(NCORES, 2, 128, NHID)
    mfull = out_arrs[j].reshape(NCORES, 128, 16)
    return (full, mfull), times


def kernel(**inputs):
    in_maps, has_gru_bias = _prep(inputs)
    (full, mfull), _ = run_prepared(in_maps, has_gru_bias, iters=1)
    res = np.empty((B, NHID), np.float32)
    mask_blk = np.empty((B, NB), np.float32)
    for c in range(NCORES):
        res[c * BL:(c + 1) * BL] = full[c].reshape(BL, NHID)
        for bt in range(2):
            mask_blk[c * BL + bt * 128: c * BL + (bt + 1) * 128] = \
                mfull[c][:, bt * 8:(bt + 1) * 8]
    mask = np.repeat(mask_blk, BS, axis=1)
    return res, mask
